# revision 51
# baseline (speedup 1.0000x reference)
"""MultiHeadAttention Trainium2 kernel (8 NeuronCores).

Sharding: core c handles batch b = c // 2 and head-group hg = c % 2
(8 of 16 heads, 512 of 1024 model dims). Attention is embarrassingly
parallel over (b, hg); the output projection is computed per head-group
against the matching W_o columns, yielding partial outputs that the host
sums (plus b_o).

Device dataflow (per core), all in "transposed" layouts so no on-device
transposes are ever needed:
  qT = Wq_hg @ Xq^T      [dh=512, S]   (lhsT = Wq_hg^T, rhs = Xq^T)
  kT = Wk_hg @ Xk^T      [dh=512, S]
  v  = Xv @ Wv_hg^T      [S, dh=512]   (+ ones column per head for softmax sums)
  scores_T[k, q] = kT_h^T-matmul, two heads PE-row-tiled concurrently
  probs = exp(scores_T / 8), one ACT instruction per head-pair (PSUM
  bank-pair read) — no max subtraction: scores ~ N(0,1), safe
  causal diag chunks: 0/1 mask multiply (post-exp) + column-trimmed
  matmuls (no memsets)
  attn_T[d, q] (+ sums row) = v^T-matmul over probs, PSUM-accumulated
  normalize: batched reciprocal, broadcast via ones-matmul, multiply
  out_partial = attn^T-matmul with Wo columns

The program is emitted as one fused pipeline: projection of block b+1
and the finalize (normalize + output projection) of query-block b-1 are
interleaved ("pumped") into attention block b's loop so the PE never
drains behind the scalar engine's exp stream.
"""

import os
from collections import deque

import numpy as np

B, S_FULL, D = 4, 2048, 1024
H, DK = 16, 64
NH_G = 8          # heads per core
DH = NH_G * DK    # 512 dims per core
P = 128
KC = 128          # key chunk (PE contraction)
SCALE = 1.0 / np.sqrt(np.float32(DK))

_PROG_CACHE = {}


def _dims(S):
    QB = min(512, S)
    return {
        "S": S, "QB": QB, "N_QB": S // QB, "N_KC": S // KC,
        "R": QB // KC, "E_CH": D // P, "M_CH": DH // P, "O_N": D // 512,
    }


def _np_dt(use_bf16):
    if use_bf16:
        import ml_dtypes
        return ml_dtypes.bfloat16
    return np.float32


class _Pump:
    """Deque of emission closures drained into another loop's gaps."""

    def __init__(self):
        self.q = deque()

    def add(self, fn):
        self.q.append(fn)

    def run(self, n):
        for _ in range(min(n, len(self.q))):
            self.q.popleft()()

    def drain(self):
        while self.q:
            self.q.popleft()()


def build_program(causal, S, use_bf16, proj_fp8=False, debug_dumps=False):
    """Build the single-core Bass/Tile program (same program on all 8 cores)."""
    from contextlib import ExitStack

    import concourse.bass as bass  # noqa: F401
    import concourse.tile as tile
    from concourse import bacc, mybir

    d = _dims(S)
    QB, N_QB, N_KC, R, E_CH, M_CH, O_N = (
        d["QB"], d["N_QB"], d["N_KC"], d["R"], d["E_CH"], d["M_CH"], d["O_N"])
    E2 = E_CH // 2      # 256-deep DoubleRow contraction chunks

    DT = mybir.dt.bfloat16 if use_bf16 else mybir.dt.float32r
    F32 = mybir.dt.float32
    F32R = mybir.dt.float32r
    FP8 = mybir.dt.float8e4
    XDT = FP8 if proj_fp8 else DT
    AF = mybir.ActivationFunctionType
    ALU = mybir.AluOpType
    DROW = mybir.MatmulPerfMode.DoubleRow

    nc = bacc.Bacc("TRN2", target_bir_lowering=False, debug=False)

    NB = S // QB
    SC_B = QB // P      # 128-row chunks per block
    xshape = [NB, P, E2, 2, QB] if proj_fp8 else [NB, P, E_CH, QB]
    wshape = [P, E2, 2, DH] if proj_fp8 else [P, E_CH, DH]
    xq_t = nc.dram_tensor("xq_t", xshape, XDT, kind="ExternalInput").ap()
    xk_t = nc.dram_tensor("xk_t", xshape, XDT, kind="ExternalInput").ap()
    xv_t = nc.dram_tensor("xv_t", xshape, XDT, kind="ExternalInput").ap()
    wq_t = nc.dram_tensor("wq_t", wshape, XDT, kind="ExternalInput").ap()
    wk_t = nc.dram_tensor("wk_t", wshape, XDT, kind="ExternalInput").ap()
    wv_t = nc.dram_tensor("wv_t", wshape, XDT, kind="ExternalInput").ap()
    wo_t = nc.dram_tensor("wo_t", [P, M_CH, D], DT,
                          kind="ExternalInput").ap()
    bq_in = nc.dram_tensor("bq_p", [P, M_CH], F32, kind="ExternalInput").ap()
    bk_in = nc.dram_tensor("bk_p", [P, M_CH], F32, kind="ExternalInput").ap()
    bv_in = nc.dram_tensor("bv_r", [P, DH], F32, kind="ExternalInput").ap()
    dmask_in = nc.dram_tensor("dmask", [P, R, QB], DT,
                              kind="ExternalInput").ap()
    ones_c_in = nc.dram_tensor("ones_c", [65, 64], F32R,
                               kind="ExternalInput").ap()
    ones_v_in = nc.dram_tensor("ones_v", [P, N_KC, NH_G, 1], DT,
                               kind="ExternalInput").ap()
    out_p = nc.dram_tensor("out_p", [S, D], DT, kind="ExternalOutput").ap()

    with tile.TileContext(nc) as tc, ExitStack() as ctx:
        consts = ctx.enter_context(tc.tile_pool(name="consts", bufs=1))
        wpool = ctx.enter_context(tc.tile_pool(name="w", bufs=1))
        qkv = ctx.enter_context(tc.tile_pool(name="qkv", bufs=1))
        xpool = ctx.enter_context(tc.tile_pool(name="xp", bufs=3))
        proj_ps = ctx.enter_context(
            tc.tile_pool(name="pj", bufs=1, space="PSUM"))
        sc_ps = ctx.enter_context(
            tc.tile_pool(name="sc", bufs=2, space="PSUM"))
        pv_ps = ctx.enter_context(
            tc.tile_pool(name="pv", bufs=2, space="PSUM"))
        fin_ps = ctx.enter_context(
            tc.tile_pool(name="fin", bufs=1, space="PSUM"))
        probs_pool = ctx.enter_context(tc.tile_pool(name="probs", bufs=6))
        aupool = ctx.enter_context(tc.tile_pool(name="au", bufs=18))
        attn_pool = ctx.enter_context(tc.tile_pool(name="attn", bufs=9))
        rbpool = ctx.enter_context(tc.tile_pool(name="rbb", bufs=2))
        sums_pool = ctx.enter_context(tc.tile_pool(name="sums", bufs=2))
        outst = ctx.enter_context(tc.tile_pool(name="outst", bufs=3))

        qT = qkv.tile([P, M_CH, S], DT, tag="qT")
        kT = qkv.tile([P, M_CH, S], DT, tag="kT")
        v_aug = qkv.tile([P, N_KC, NH_G, 65], DT, tag="v_aug")

        def load_consts():
            """Emit const/weight DMAs that are not needed immediately; they
            ride the gpsimd DMA ring, parallel to the x loads on sync."""
            nc.gpsimd.dma_start(bq_sb, bq_in)
            nc.gpsimd.dma_start(bk_sb, bk_in)
            nc.gpsimd.dma_start(bv_sb, bv_in)
            nc.gpsimd.dma_start(dmask, dmask_in)
            nc.gpsimd.dma_start(ones_c, ones_c_in)
            if use_bf16:
                nc.gpsimd.memset(v_aug[:, :, :, 64:65], 1.0)
            else:
                nc.gpsimd.dma_start(v_aug[:, :, :, 64:65], ones_v_in)
            w_sb = wpool.tile([P, M_CH, D], DT, tag="wo")
            nc.gpsimd.dma_start(w_sb, wo_t)
            return w_sb

        bq_sb = consts.tile([P, M_CH], F32, tag="bq")
        bk_sb = consts.tile([P, M_CH], F32, tag="bk")
        bv_sb = consts.tile([P, DH], F32, tag="bv")
        dmask = consts.tile([P, R, QB], DT, tag="dmask")
        ones_c = consts.tile([65, 64], F32R, tag="ones_c")

        w_tiles = {}
        wtile_shape = [P, E2, 2, DH] if proj_fp8 else [P, E_CH, DH]
        W_SRC = {"wk": wk_t, "wv": wv_t, "wq": wq_t}

        PJW = max(QB, DH)
        PHASES = (("k", xk_t, "wk"), ("v", xv_t, "wv"), ("q", xq_t, "wq"))

        xtile_shape = [P, E2, 2, QB] if proj_fp8 else [P, E_CH, QB]

        def proj_prefetch(blk, load_w=False):
            """Issue the x (and for block 0, interleaved weight) DMAs so the
            first projection matmul's dependencies sit at the queue head.
            Block 0's k-phase tensors are split per contraction chunk so the
            first matmul starts after ~200KB instead of 1.5MB; the other
            weights ride the gpsimd DMA ring in parallel."""
            xhs = {}
            for phase, x_in, wname in PHASES:
                xblk = xpool.tile(xtile_shape, XDT, tag="x",
                                  name=f"x{phase}{blk}")
                if load_w:
                    w_sb = wpool.tile(wtile_shape, XDT, tag=wname, name=wname)
                    if phase == "k":
                        for e in range(wtile_shape[1]):
                            nc.sync.dma_start(w_sb[:, e], W_SRC[wname][:, e])
                            nc.sync.dma_start(xblk[:, e], x_in[blk, :, e])
                    else:
                        nc.gpsimd.dma_start(w_sb, W_SRC[wname])
                        nc.sync.dma_start(xblk, x_in[blk])
                    w_tiles[wname] = w_sb
                else:
                    nc.sync.dma_start(xblk, x_in[blk])
                xhs[phase] = xblk
            return xhs

        def proj_steps(blk, xhs, pump, alt_bank=False):
            """Append projection emission steps for s-block `blk`. With
            alt_bank (block 0, before attention exists), alternate chains
            into the idle fin bank so accumulation double-buffers."""
            cnt = {"i": 0}

            def pj_alloc(ph):
                tag = "fin" if alt_bank and cnt["i"] % 2 else "pj"
                pool = fin_ps if tag == "fin" else proj_ps
                cnt["i"] += 1
                ph["ps"] = pool.tile([P, PJW], F32, tag=tag, name="pjt")

            for phase, x_in, wname in PHASES:
                w_sb = w_tiles[wname]
                xh = {"x": xhs[phase]}

                n_mm = E2 if proj_fp8 else E_CH
                if phase in ("q", "k"):
                    b_sb = bq_sb if phase == "q" else bk_sb
                    dstp = qT if phase == "q" else kT
                    for m in range(M_CH):
                        ph = {}

                        def alloc_step(ph=ph):
                            pj_alloc(ph)
                        pump.add(alloc_step)
                        for e in range(n_mm):
                            def mm_step(m=m, e=e, w_sb=w_sb, ph=ph, xh=xh,
                                        n_mm=n_mm):
                                if proj_fp8:
                                    nc.tensor.matmul(
                                        ph["ps"][:, 0:QB],
                                        lhsT=w_sb[:, e, :, m * P:(m + 1) * P],
                                        rhs=xh["x"][:, e],
                                        start=(e == 0), stop=(e == n_mm - 1),
                                        perf_mode=DROW,
                                    )
                                else:
                                    nc.tensor.matmul(
                                        ph["ps"][:, 0:QB],
                                        lhsT=w_sb[:, e, m * P:(m + 1) * P],
                                        rhs=xh["x"][:, e, :],
                                        start=(e == 0), stop=(e == n_mm - 1),
                                    )
                            pump.add(mm_step)

                        def drain_step(m=m, dstp=dstp, b_sb=b_sb, ph=ph):
                            sl = slice(blk * QB, (blk + 1) * QB)
                            nc.vector.tensor_scalar_add(
                                dstp[:, m, sl], ph["ps"][:, 0:QB],
                                b_sb[:, m:m + 1])
                        pump.add(drain_step)
                else:
                    for sc in range(SC_B):
                        ph = {}

                        def alloc_step(ph=ph):
                            pj_alloc(ph)
                        pump.add(alloc_step)
                        for e in range(n_mm):
                            def mm_step(sc=sc, e=e, w_sb=w_sb, ph=ph, xh=xh,
                                        n_mm=n_mm):
                                if proj_fp8:
                                    nc.tensor.matmul(
                                        ph["ps"][:, 0:DH],
                                        lhsT=xh["x"][:, e, :,
                                                     sc * P:(sc + 1) * P],
                                        rhs=w_sb[:, e],
                                        start=(e == 0), stop=(e == n_mm - 1),
                                        perf_mode=DROW,
                                    )
                                else:
                                    nc.tensor.matmul(
                                        ph["ps"][:, 0:DH],
                                        lhsT=xh["x"][:, e, sc * P:(sc + 1) * P],
                                        rhs=w_sb[:, e, :],
                                        start=(e == 0), stop=(e == n_mm - 1),
                                    )
                            pump.add(mm_step)

                        def drain_step(sc=sc, ph=ph):
                            kc = blk * SC_B + sc
                            nc.vector.tensor_tensor(
                                v_aug[:, kc, :, 0:64],
                                ph["ps"][:, 0:DH].rearrange(
                                    "p (h e) -> p h e", h=NH_G),
                                bv_sb.rearrange("p (h e) -> p h e", h=NH_G),
                                ALU.add,
                            )
                        pump.add(drain_step)

        def make_fin(qb, au_tiles, sums_t):
            """Finalize-qb emitters: reciprocal, broadcast+normalize,
            out-projection. Returned closures are composed either via the
            late pump (steady state) or eagerly (last query-block)."""
            st8 = {"recips": [None, None], "attn": {}}

            def recip_half(half):
                # column-chunked so each DVE reciprocal stays ~1us and the
                # attention mask multiplies can slot in between
                rt = sums_pool.tile([97, QB], F32R, tag=f"rec{half}",
                                    name=f"rec{half}")
                with nc.allow_low_precision(
                        reason="softmax denom recip, f32r rounding"):
                    for c in range(0, QB, KC):
                        nc.vector.reciprocal(rt[:, c:c + KC],
                                             sums_t[half][:, c:c + KC])
                st8["recips"][half] = rt

            def rb_one(m):
                attn_m = attn_pool.tile([P, QB], DT, tag="attn",
                                        name="attn_m")
                for hl in (0, 1):
                    idx = 2 * m + hl
                    half, j = divmod(idx, 4)
                    recip65 = rbpool.tile([65, QB], F32R, tag="rbb",
                                          name="recip65")
                    nc.sync.dma_start(
                        recip65[64:65, :],
                        st8["recips"][half][32 * j:32 * j + 1, :])
                    rb = fin_ps.tile([64, QB], F32, tag="fin", name="rb")
                    nc.tensor.matmul(rb, lhsT=ones_c[64:65, :],
                                     rhs=recip65[64:65, :],
                                     start=True, stop=True)
                    nc.vector.tensor_tensor(
                        attn_m[64 * hl:64 * hl + 64, :],
                        au_tiles[idx][0:64, :], rb, ALU.mult)
                st8["attn"][m] = attn_m

            def op_steps(pump, alt_bank=False):
                for i, (ssub, nout) in enumerate(
                        (s, n) for s in range(QB // P) for n in range(O_N)):
                        oh = {}
                        # on the last query-block the proj bank is idle:
                        # alternate into it so out-proj groups double-buffer
                        tag = "pj" if alt_bank and i % 2 else "fin"
                        pool = proj_ps if tag == "pj" else fin_ps

                        def op_mm_step(ssub=ssub, nout=nout, oh=oh,
                                       tag=tag, pool=pool):
                            pso = pool.tile([P, 512], F32, tag=tag,
                                            name="pso")
                            for m in range(M_CH):
                                nc.tensor.matmul(
                                    pso,
                                    lhsT=st8["attn"][m][:,
                                                        ssub * P:(ssub + 1) * P],
                                    rhs=wo_sb[:, m,
                                              nout * 512:(nout + 1) * 512],
                                    start=(m == 0), stop=(m == M_CH - 1),
                                )
                            oh["pso"] = pso
                        pump.add(op_mm_step)

                        def op_st_step(ssub=ssub, nout=nout, oh=oh):
                            st = outst.tile([P, 512], DT, tag="st",
                                            name="st")
                            nc.vector.tensor_copy(st, oh["pso"])
                            r0 = qb * QB + ssub * P
                            nc.gpsimd.dma_start(
                                out_p[r0:r0 + P,
                                      nout * 512:(nout + 1) * 512], st)
                        pump.add(op_st_step)

            return recip_half, rb_one, op_steps

        def fin_steps(qb, au_tiles, sums_t, pump):
            recip_half, rb_one, op_steps = make_fin(qb, au_tiles, sums_t)
            pump.add(lambda: recip_half(0))
            pump.add(lambda: recip_half(1))
            for m in range(M_CH):
                pump.add(lambda m=m: rb_one(m))
            op_steps(pump)

        # ---- fused pipeline ----
        pump = _Pump()      # projection work for the next block
        late = _Pump()      # finalize work for the previous query-block
        xhs = proj_prefetch(0, load_w=True)
        wo_sb = load_consts()
        proj_steps(0, xhs, pump, alt_bank=True)
        pump.drain()

        prev_fin = None     # (qb, au_tiles, sums_t) awaiting finalize
        for qb in range(N_QB):
            if qb + 1 < N_QB:
                xhs = proj_prefetch(qb + 1)
                proj_steps(qb + 1, xhs, pump)
            if prev_fin is not None:
                fin_steps(*prev_fin, late)

            n_kc = (qb + 1) * R if causal else N_KC
            slots = n_kc * M_CH
            lslots = max(1, n_kc * (M_CH - 1))
            au_tiles = {}
            sums_t = [sums_pool.tile([97, QB], F32, tag=f"sums{h}",
                                     name=f"sums{h}") for h in (0, 1)]
            for h in (0, 1):
                nc.gpsimd.memset(sums_t[h], 1.0)
            is_last = qb == N_QB - 1
            if is_last:
                eager_fin = make_fin(qb, au_tiles, sums_t)
            for m in range(M_CH):
                if is_last and m == 2:
                    # sums rows for m0/m1 are in flight: reciprocal half 0
                    # and their normalizes overlap the m2/m3 attention.
                    eager_fin[0](0)
                    eager_fin[1](0)
                    eager_fin[1](1)
                pv_t = [pv_ps.tile([65, QB], F32, tag="pv", name=f"pv{hl}")
                        for hl in (0, 1)]

                def emit_pv(pt, c0, kc):
                    for hl in (0, 1):
                        nc.tensor.matmul(
                            pv_t[hl][:, c0:],
                            lhsT=v_aug[:, kc, 2 * m + hl, :],
                            rhs=pt[:, hl, c0:],
                            start=(kc == 0), stop=(kc == n_kc - 1),
                        )

                pending = None   # chunk whose PV is deferred one iteration
                for kc in range(n_kc):
                    # spread pending proj work over the whole loop; the
                    # finalize of qb-1 waits until m >= 1 (its reciprocal
                    # needs qb-1's sums DMAs, which land around m0).
                    pump.run(-(-len(pump.q) // slots))
                    slots -= 1
                    if m >= 1:
                        late.run(-(-len(late.q) // lslots))
                        lslots -= 1

                    r = kc - (n_kc - R)
                    is_diag = causal and r >= 0
                    c0 = KC * r if is_diag and r > 0 else 0
                    sc = sc_ps.tile([P, 2, QB], F32, tag="sc", name="sct")
                    for hl in (0, 1):
                        rows = slice(64 * hl, 64 * hl + 64)
                        nc.tensor.matmul(
                            sc[:, hl, c0:],
                            lhsT=kT[rows, m, kc * KC:(kc + 1) * KC],
                            rhs=qT[rows, m, qb * QB + c0:(qb + 1) * QB],
                            start=True, stop=True,
                        )
                    pt = probs_pool.tile([P, 2, QB], DT, tag="pt", name="pt")
                    nc.scalar.activation(pt[:, :, c0:], sc[:, :, c0:],
                                         AF.Exp, scale=float(SCALE))
                    if is_diag:
                        # only the 128-wide band [c0, c0+KC) is partially
                        # masked; columns beyond it are fully visible
                        for hl in (0, 1):
                            nc.vector.tensor_tensor(
                                pt[:, hl, c0:c0 + KC], pt[:, hl, c0:c0 + KC],
                                dmask[:, r, c0:c0 + KC], ALU.mult)
                    # software pipelining: this chunk's PV is emitted after
                    # the NEXT chunk's scores, so the PE queue never
                    # head-of-line blocks on the exp that PV depends on
                    if pending is not None:
                        emit_pv(*pending)
                    pending = (pt, c0, kc)
                emit_pv(*pending)
                for hl in (0, 1):
                    idx = 2 * m + hl
                    au = aupool.tile([65, QB], F32, tag="au",
                                     name=f"au{idx}")
                    nc.vector.tensor_copy(au, pv_t[hl])
                    au_tiles[idx] = au
                    half, j = divmod(idx, 4)
                    nc.sync.dma_start(sums_t[half][32 * j:32 * j + 1, :],
                                      au[64:65, :])
            pump.drain()
            late.drain()
            prev_fin = (qb, au_tiles, sums_t)

        # last query-block: m0/m1 were normalized eagerly inside the loop
        recip_half, rb_one, op_steps = eager_fin
        recip_half(1)
        rb_one(2)
        rb_one(3)
        op_steps(late, alt_bank=True)
        late.drain()

    nc.compile()
    return nc


def make_consts(S, use_bf16):
    """Host-built 0/1 causal masks for the R diagonal key-chunks."""
    d = _dims(S)
    QB, R = d["QB"], d["R"]
    npdt = _np_dt(use_bf16)
    i = np.arange(P)[:, None]
    j = np.arange(QB)[None, :]
    dmask = np.stack([(i <= j - KC * r) for r in range(R)], 1)
    return dmask.astype(npdt)


def core_inputs(Q, K, V, W_q, b_q, W_k, b_k, W_v, b_v, W_o, b, hg, S, use_bf16,
                proj_fp8=False):
    """Build the per-core input map (host-side slicing/transposition/casts)."""
    npdt = _np_dt(use_bf16)
    d = _dims(S)
    M_CH = d["M_CH"]
    rows = slice(hg * DH, (hg + 1) * DH)

    QB = d["QB"]
    E_CH = D // P
    E2 = E_CH // 2
    if proj_fp8:
        import ml_dtypes
        fp8dt = ml_dtypes.float8_e4m3fn

    def xt(x):
        # [S, D] -> [N_QB, P, E_CH, QB] (bf16) or [N_QB, P, E2, 2, QB] (fp8
        # DoubleRow pairs): transposed/tiled so each block load is one DMA.
        a = np.asarray(x, np.float32).T                   # [D, S]
        if proj_fp8:
            a = a.astype(fp8dt)
            a = a.reshape(E2, 2, P, S // QB, QB).transpose(3, 2, 0, 1, 4)
        else:
            a = a.astype(npdt)
            a = a.reshape(E_CH, P, S // QB, QB).transpose(2, 1, 0, 3)
        return np.ascontiguousarray(a)

    def wt(w):
        # [DH, D] slice -> W^T tiled [P, E_CH, DH] / [P, E2, 2, DH]
        a = np.asarray(w, np.float32).T                   # [D, DH]
        if proj_fp8:
            a = a.astype(fp8dt)
            return np.ascontiguousarray(
                a.reshape(E2, 2, P, DH).transpose(2, 0, 1, 3))
        a = a.astype(npdt)
        return np.ascontiguousarray(
            a.reshape(E_CH, P, DH).transpose(1, 0, 2))

    a_wo = np.asarray(W_o[:, rows], np.float32).T.astype(npdt)  # [DH, D]
    wo_prep = np.ascontiguousarray(
        a_wo.reshape(M_CH, P, D).transpose(1, 0, 2))

    dmask = make_consts(S, use_bf16)
    return {
        "xq_t": xt(Q[b]), "xk_t": xt(K[b]), "xv_t": xt(V[b]),
        "wq_t": wt(W_q[rows]), "wk_t": wt(W_k[rows]), "wv_t": wt(W_v[rows]),
        "wo_t": wo_prep,
        "bq_p": np.ascontiguousarray(
            np.asarray(b_q[rows], np.float32).reshape(M_CH, P).T),
        "bk_p": np.ascontiguousarray(
            np.asarray(b_k[rows], np.float32).reshape(M_CH, P).T),
        "bv_r": np.broadcast_to(
            np.asarray(b_v[rows], np.float32), (P, DH)).copy(),
        "dmask": dmask,
        "ones_c": np.ones((65, 64), np.float32),
        "ones_v": np.ones((P, d["N_KC"], NH_G, 1), npdt),
    }


def _np_reference(Q, K, V, mask, W_q, b_q, W_k, b_k, W_v, b_v, W_o, b_o):
    """Exact numpy fallback for arbitrary masks."""
    q = (Q @ W_q.T + b_q).reshape(B, S_FULL, H, DK).transpose(0, 2, 1, 3)
    k = (K @ W_k.T + b_k).reshape(B, S_FULL, H, DK).transpose(0, 2, 1, 3)
    v = (V @ W_v.T + b_v).reshape(B, S_FULL, H, DK).transpose(0, 2, 1, 3)
    scores = np.einsum("bhqd,bhkd->bhqk", q, k) / np.sqrt(np.float32(DK))
    scores = np.where(mask == 0, np.finfo(np.float32).min, scores)
    scores -= scores.max(-1, keepdims=True)
    probs = np.exp(scores)
    probs /= probs.sum(-1, keepdims=True)
    out = np.einsum("bhqk,bhkd->bhqd", probs, v)
    out = out.transpose(0, 2, 1, 3).reshape(B, S_FULL, D)
    return (out @ W_o.T + b_o).astype(np.float32)


def kernel(Q, K, V, mask, W_q, b_q, W_k, b_k, W_v, b_v, W_o, b_o):
    Q = np.asarray(Q, np.float32)
    K = np.asarray(K, np.float32)
    V = np.asarray(V, np.float32)
    mask = np.asarray(mask)

    m2 = mask.reshape(mask.shape[-2], mask.shape[-1])
    if np.array_equal(m2 != 0, np.tril(np.ones(m2.shape, bool))):
        causal = True
    elif (m2 != 0).all():
        causal = False
    else:
        return _np_reference(Q, K, V, mask, W_q, b_q, W_k, b_k, W_v, b_v,
                             W_o, b_o)

    use_bf16 = os.environ.get("MHA_KERNEL_DTYPE", "bf16") == "bf16"
    proj_fp8 = os.environ.get("MHA_PROJ_FP8", "0") == "1"
    from concourse.bass_utils import run_bass_kernel_spmd

    key = (causal, S_FULL, use_bf16, proj_fp8)
    if key not in _PROG_CACHE:
        _PROG_CACHE[key] = build_program(causal, S_FULL, use_bf16, proj_fp8)
    nc = _PROG_CACHE[key]

    in_maps = []
    for c in range(8):
        b, hg = divmod(c, 2)
        in_maps.append(core_inputs(Q, K, V, W_q, b_q, W_k, b_k, W_v, b_v,
                                   W_o, b, hg, S_FULL, use_bf16, proj_fp8))

    trace = os.environ.get("MHA_KERNEL_TRACE", "0") == "1"
    kw = {}
    if trace:
        kw = {"trace": True,
              "trace_cores": [int(x) for x in os.environ.get(
                  "MHA_TRACE_CORES", "0").split(",")]}
    n_cores = int(os.environ.get("MHA_CORES", "8"))
    res = run_bass_kernel_spmd(nc, in_maps[:n_cores],
                               core_ids=list(range(n_cores)), **kw)
    kernel.last_results = res

    b_o32 = np.asarray(b_o, np.float32)
    out = np.zeros((B, S_FULL, D), np.float32)
    for b in range(B):
        if 2 * b + 1 < n_cores:
            out[b] = (np.asarray(res.results[2 * b]["out_p"], np.float32)
                      + np.asarray(res.results[2 * b + 1]["out_p"],
                                   np.float32) + b_o32[None, :])
    return out


kernel.last_results = None


# revision 54
# speedup vs baseline: 1.0111x; 1.0111x over previous
"""MultiHeadAttention Trainium2 kernel (8 NeuronCores).

Sharding: core c handles batch b = c // 2 and head-group hg = c % 2
(8 of 16 heads, 512 of 1024 model dims). Attention is embarrassingly
parallel over (b, hg); the output projection is computed per head-group
against the matching W_o columns, yielding partial outputs that the host
sums (plus b_o).

Device dataflow (per core), all in "transposed" layouts so no on-device
transposes are ever needed:
  qT = Wq_hg @ Xq^T      [dh=512, S]   (lhsT = Wq_hg^T, rhs = Xq^T)
  kT = Wk_hg @ Xk^T      [dh=512, S]
  v  = Xv @ Wv_hg^T      [S, dh=512]   (+ ones column per head for softmax sums)
  scores_T[k, q] = kT_h^T-matmul, two heads PE-row-tiled concurrently
  probs = exp(scores_T / 8), one ACT instruction per head-pair (PSUM
  bank-pair read) — no max subtraction: scores ~ N(0,1), safe
  causal diag chunks: 0/1 mask multiply (post-exp) + column-trimmed
  matmuls (no memsets)
  attn_T[d, q] (+ sums row) = v^T-matmul over probs, PSUM-accumulated
  normalize: batched reciprocal, broadcast via ones-matmul, multiply
  out_partial = attn^T-matmul with Wo columns

The program is emitted as one fused pipeline: projection of block b+1
and the finalize (normalize + output projection) of query-block b-1 are
interleaved ("pumped") into attention block b's loop so the PE never
drains behind the scalar engine's exp stream.
"""

import os
from collections import deque

import numpy as np

B, S_FULL, D = 4, 2048, 1024
H, DK = 16, 64
NH_G = 8          # heads per core
DH = NH_G * DK    # 512 dims per core
P = 128
KC = 128          # key chunk (PE contraction)
SCALE = 1.0 / np.sqrt(np.float32(DK))

_PROG_CACHE = {}


def _dims(S):
    QB = min(512, S)
    return {
        "S": S, "QB": QB, "N_QB": S // QB, "N_KC": S // KC,
        "R": QB // KC, "E_CH": D // P, "M_CH": DH // P, "O_N": D // 512,
    }


def _np_dt(use_bf16):
    if use_bf16:
        import ml_dtypes
        return ml_dtypes.bfloat16
    return np.float32


class _Pump:
    """Deque of emission closures drained into another loop's gaps."""

    def __init__(self):
        self.q = deque()

    def add(self, fn):
        self.q.append(fn)

    def run(self, n):
        for _ in range(min(n, len(self.q))):
            self.q.popleft()()

    def drain(self):
        while self.q:
            self.q.popleft()()


def build_program(causal, S, use_bf16, proj_fp8=False, debug_dumps=False):
    """Build the single-core Bass/Tile program (same program on all 8 cores)."""
    from contextlib import ExitStack

    import concourse.bass as bass  # noqa: F401
    import concourse.tile as tile
    from concourse import bacc, mybir

    d = _dims(S)
    QB, N_QB, N_KC, R, E_CH, M_CH, O_N = (
        d["QB"], d["N_QB"], d["N_KC"], d["R"], d["E_CH"], d["M_CH"], d["O_N"])
    E2 = E_CH // 2      # 256-deep DoubleRow contraction chunks

    DT = mybir.dt.bfloat16 if use_bf16 else mybir.dt.float32r
    F32 = mybir.dt.float32
    F32R = mybir.dt.float32r
    FP8 = mybir.dt.float8e4
    XDT = FP8 if proj_fp8 else DT
    AF = mybir.ActivationFunctionType
    ALU = mybir.AluOpType
    DROW = mybir.MatmulPerfMode.DoubleRow

    nc = bacc.Bacc("TRN2", target_bir_lowering=False, debug=False)

    NB = S // QB
    SC_B = QB // P      # 128-row chunks per block
    xshape = [NB, P, E2, 2, QB] if proj_fp8 else [NB, P, E_CH, QB]
    wshape = [P, E2, 2, DH] if proj_fp8 else [P, E_CH, DH]
    xq_t = nc.dram_tensor("xq_t", xshape, XDT, kind="ExternalInput").ap()
    xk_t = nc.dram_tensor("xk_t", xshape, XDT, kind="ExternalInput").ap()
    xv_t = nc.dram_tensor("xv_t", xshape, XDT, kind="ExternalInput").ap()
    wq_t = nc.dram_tensor("wq_t", wshape, XDT, kind="ExternalInput").ap()
    wk_t = nc.dram_tensor("wk_t", wshape, XDT, kind="ExternalInput").ap()
    wv_t = nc.dram_tensor("wv_t", wshape, XDT, kind="ExternalInput").ap()
    wo_t = nc.dram_tensor("wo_t", [P, M_CH, D], DT,
                          kind="ExternalInput").ap()
    bq_in = nc.dram_tensor("bq_p", [P, M_CH], F32, kind="ExternalInput").ap()
    bk_in = nc.dram_tensor("bk_p", [P, M_CH], F32, kind="ExternalInput").ap()
    bv_in = nc.dram_tensor("bv_r", [P, DH], F32, kind="ExternalInput").ap()
    dmask_in = nc.dram_tensor("dmask", [P, R, QB], DT,
                              kind="ExternalInput").ap()
    ones_c_in = nc.dram_tensor("ones_c", [65, 64], F32R,
                               kind="ExternalInput").ap()
    ones_v_in = nc.dram_tensor("ones_v", [P, N_KC, NH_G, 1], DT,
                               kind="ExternalInput").ap()
    out_p = nc.dram_tensor("out_p", [S, D], DT, kind="ExternalOutput").ap()

    with tile.TileContext(nc) as tc, ExitStack() as ctx:
        consts = ctx.enter_context(tc.tile_pool(name="consts", bufs=1))
        wpool = ctx.enter_context(tc.tile_pool(name="w", bufs=1))
        qkv = ctx.enter_context(tc.tile_pool(name="qkv", bufs=1))
        xpool = ctx.enter_context(tc.tile_pool(name="xp", bufs=3))
        proj_ps = ctx.enter_context(
            tc.tile_pool(name="pj", bufs=1, space="PSUM"))
        sc_ps = ctx.enter_context(
            tc.tile_pool(name="sc", bufs=2, space="PSUM"))
        pv_ps = ctx.enter_context(
            tc.tile_pool(name="pv", bufs=2, space="PSUM"))
        fin_ps = ctx.enter_context(
            tc.tile_pool(name="fin", bufs=1, space="PSUM"))
        probs_pool = ctx.enter_context(tc.tile_pool(name="probs", bufs=6))
        aupool = ctx.enter_context(tc.tile_pool(name="au", bufs=18))
        attn_pool = ctx.enter_context(tc.tile_pool(name="attn", bufs=9))
        rbpool = ctx.enter_context(tc.tile_pool(name="rbb", bufs=2))
        sums_pool = ctx.enter_context(tc.tile_pool(name="sums", bufs=2))
        outst = ctx.enter_context(tc.tile_pool(name="outst", bufs=3))

        qT = qkv.tile([P, M_CH, S], DT, tag="qT")
        kT = qkv.tile([P, M_CH, S], DT, tag="kT")
        v_aug = qkv.tile([P, N_KC, NH_G, 65], DT, tag="v_aug")

        def load_consts():
            """Emit const/weight DMAs that are not needed immediately; they
            ride the gpsimd DMA ring, parallel to the x loads on sync."""
            nc.gpsimd.dma_start(bq_sb, bq_in)
            nc.gpsimd.dma_start(bk_sb, bk_in)
            nc.gpsimd.dma_start(bv_sb, bv_in)
            nc.gpsimd.dma_start(dmask, dmask_in)
            nc.gpsimd.dma_start(ones_c, ones_c_in)
            if use_bf16:
                nc.gpsimd.memset(v_aug[:, :, :, 64:65], 1.0)
            else:
                nc.gpsimd.dma_start(v_aug[:, :, :, 64:65], ones_v_in)
            w_sb = wpool.tile([P, M_CH, D], DT, tag="wo")
            nc.gpsimd.dma_start(w_sb, wo_t)
            return w_sb

        bq_sb = consts.tile([P, M_CH], F32, tag="bq")
        bk_sb = consts.tile([P, M_CH], F32, tag="bk")
        bv_sb = consts.tile([P, DH], F32, tag="bv")
        dmask = consts.tile([P, R, QB], DT, tag="dmask")
        ones_c = consts.tile([65, 64], F32R, tag="ones_c")

        w_tiles = {}
        wtile_shape = [P, E2, 2, DH] if proj_fp8 else [P, E_CH, DH]
        W_SRC = {"wk": wk_t, "wv": wv_t, "wq": wq_t}

        PJW = max(QB, DH)
        PHASES = (("k", xk_t, "wk"), ("v", xv_t, "wv"), ("q", xq_t, "wq"))

        xtile_shape = [P, E2, 2, QB] if proj_fp8 else [P, E_CH, QB]

        def proj_prefetch(blk, load_w=False):
            """Issue the x (and for block 0, interleaved weight) DMAs so the
            first projection matmul's dependencies sit at the queue head.
            Block 0's k-phase tensors are split per contraction chunk so the
            first matmul starts after ~200KB instead of 1.5MB; the other
            weights ride the gpsimd DMA ring in parallel."""
            w_eng = {"k": nc.sync, "v": nc.gpsimd, "q": nc.gpsimd}
            x_eng = {"k": nc.scalar, "v": nc.sync, "q": nc.scalar}
            xhs = {}
            for phase, x_in, wname in PHASES:
                xblk = xpool.tile(xtile_shape, XDT, tag="x",
                                  name=f"x{phase}{blk}")
                if load_w:
                    w_sb = wpool.tile(wtile_shape, XDT, tag=wname, name=wname)
                    w_eng[phase].dma_start(w_sb, W_SRC[wname])
                    x_eng[phase].dma_start(xblk, x_in[blk])
                    w_tiles[wname] = w_sb
                else:
                    nc.sync.dma_start(xblk, x_in[blk])
                xhs[phase] = xblk
            return xhs

        def proj_steps(blk, xhs, pump, alt_bank=False):
            """Append projection emission steps for s-block `blk`. With
            alt_bank (block 0, before attention exists), alternate chains
            into the idle fin bank so accumulation double-buffers."""
            cnt = {"i": 0}

            def pj_alloc(ph):
                tag = "fin" if alt_bank and cnt["i"] % 2 else "pj"
                pool = fin_ps if tag == "fin" else proj_ps
                cnt["i"] += 1
                ph["ps"] = pool.tile([P, PJW], F32, tag=tag, name="pjt")

            for phase, x_in, wname in PHASES:
                w_sb = w_tiles[wname]
                xh = {"x": xhs[phase]}

                n_mm = E2 if proj_fp8 else E_CH
                if phase in ("q", "k"):
                    b_sb = bq_sb if phase == "q" else bk_sb
                    dstp = qT if phase == "q" else kT
                    for m in range(M_CH):
                        ph = {}

                        def alloc_step(ph=ph):
                            pj_alloc(ph)
                        pump.add(alloc_step)
                        for e in range(n_mm):
                            def mm_step(m=m, e=e, w_sb=w_sb, ph=ph, xh=xh,
                                        n_mm=n_mm):
                                if proj_fp8:
                                    nc.tensor.matmul(
                                        ph["ps"][:, 0:QB],
                                        lhsT=w_sb[:, e, :, m * P:(m + 1) * P],
                                        rhs=xh["x"][:, e],
                                        start=(e == 0), stop=(e == n_mm - 1),
                                        perf_mode=DROW,
                                    )
                                else:
                                    nc.tensor.matmul(
                                        ph["ps"][:, 0:QB],
                                        lhsT=w_sb[:, e, m * P:(m + 1) * P],
                                        rhs=xh["x"][:, e, :],
                                        start=(e == 0), stop=(e == n_mm - 1),
                                    )
                            pump.add(mm_step)

                        def drain_step(m=m, dstp=dstp, b_sb=b_sb, ph=ph):
                            sl = slice(blk * QB, (blk + 1) * QB)
                            nc.vector.tensor_scalar_add(
                                dstp[:, m, sl], ph["ps"][:, 0:QB],
                                b_sb[:, m:m + 1])
                        pump.add(drain_step)
                else:
                    for sc in range(SC_B):
                        ph = {}

                        def alloc_step(ph=ph):
                            pj_alloc(ph)
                        pump.add(alloc_step)
                        for e in range(n_mm):
                            def mm_step(sc=sc, e=e, w_sb=w_sb, ph=ph, xh=xh,
                                        n_mm=n_mm):
                                if proj_fp8:
                                    nc.tensor.matmul(
                                        ph["ps"][:, 0:DH],
                                        lhsT=xh["x"][:, e, :,
                                                     sc * P:(sc + 1) * P],
                                        rhs=w_sb[:, e],
                                        start=(e == 0), stop=(e == n_mm - 1),
                                        perf_mode=DROW,
                                    )
                                else:
                                    nc.tensor.matmul(
                                        ph["ps"][:, 0:DH],
                                        lhsT=xh["x"][:, e, sc * P:(sc + 1) * P],
                                        rhs=w_sb[:, e, :],
                                        start=(e == 0), stop=(e == n_mm - 1),
                                    )
                            pump.add(mm_step)

                        def drain_step(sc=sc, ph=ph):
                            kc = blk * SC_B + sc
                            nc.vector.tensor_tensor(
                                v_aug[:, kc, :, 0:64],
                                ph["ps"][:, 0:DH].rearrange(
                                    "p (h e) -> p h e", h=NH_G),
                                bv_sb.rearrange("p (h e) -> p h e", h=NH_G),
                                ALU.add,
                            )
                        pump.add(drain_step)

        def make_fin(qb, au_tiles, sums_t):
            """Finalize-qb emitters: reciprocal, broadcast+normalize,
            out-projection. Returned closures are composed either via the
            late pump (steady state) or eagerly (last query-block)."""
            st8 = {"recips": [None, None], "attn": {}}

            def recip_half(half):
                # column-chunked so each DVE reciprocal stays ~1us and the
                # attention mask multiplies can slot in between
                rt = sums_pool.tile([97, QB], F32R, tag=f"rec{half}",
                                    name=f"rec{half}")
                with nc.allow_low_precision(
                        reason="softmax denom recip, f32r rounding"):
                    for c in range(0, QB, KC):
                        nc.vector.reciprocal(rt[:, c:c + KC],
                                             sums_t[half][:, c:c + KC])
                st8["recips"][half] = rt

            def rb_one(m):
                attn_m = attn_pool.tile([P, QB], DT, tag="attn",
                                        name="attn_m")
                for hl in (0, 1):
                    idx = 2 * m + hl
                    half, j = divmod(idx, 4)
                    recip65 = rbpool.tile([65, QB], F32R, tag="rbb",
                                          name="recip65")
                    nc.sync.dma_start(
                        recip65[64:65, :],
                        st8["recips"][half][32 * j:32 * j + 1, :])
                    rb = fin_ps.tile([64, QB], F32, tag="fin", name="rb")
                    nc.tensor.matmul(rb, lhsT=ones_c[64:65, :],
                                     rhs=recip65[64:65, :],
                                     start=True, stop=True)
                    nc.vector.tensor_tensor(
                        attn_m[64 * hl:64 * hl + 64, :],
                        au_tiles[idx][0:64, :], rb, ALU.mult)
                st8["attn"][m] = attn_m

            def op_steps(pump, alt_bank=False):
                for i, (ssub, nout) in enumerate(
                        (s, n) for s in range(QB // P) for n in range(O_N)):
                        oh = {}
                        # on the last query-block the proj bank is idle:
                        # alternate into it so out-proj groups double-buffer
                        tag = "pj" if alt_bank and i % 2 else "fin"
                        pool = proj_ps if tag == "pj" else fin_ps

                        def op_mm_step(ssub=ssub, nout=nout, oh=oh,
                                       tag=tag, pool=pool):
                            pso = pool.tile([P, 512], F32, tag=tag,
                                            name="pso")
                            for m in range(M_CH):
                                nc.tensor.matmul(
                                    pso,
                                    lhsT=st8["attn"][m][:,
                                                        ssub * P:(ssub + 1) * P],
                                    rhs=wo_sb[:, m,
                                              nout * 512:(nout + 1) * 512],
                                    start=(m == 0), stop=(m == M_CH - 1),
                                )
                            oh["pso"] = pso
                        pump.add(op_mm_step)

                        def op_st_step(ssub=ssub, nout=nout, oh=oh, i=i):
                            st = outst.tile([P, 512], DT, tag="st",
                                            name="st")
                            nc.vector.tensor_copy(st, oh["pso"])
                            r0 = qb * QB + ssub * P
                            eng = nc.gpsimd if i % 2 else nc.sync
                            eng.dma_start(
                                out_p[r0:r0 + P,
                                      nout * 512:(nout + 1) * 512], st)
                        pump.add(op_st_step)

            return recip_half, rb_one, op_steps

        def fin_steps(qb, au_tiles, sums_t, pump):
            recip_half, rb_one, op_steps = make_fin(qb, au_tiles, sums_t)
            pump.add(lambda: recip_half(0))
            pump.add(lambda: recip_half(1))
            for m in range(M_CH):
                pump.add(lambda m=m: rb_one(m))
            op_steps(pump)

        # ---- fused pipeline ----
        pump = _Pump()      # projection work for the next block
        late = _Pump()      # finalize work for the previous query-block
        xhs = proj_prefetch(0, load_w=True)
        wo_sb = load_consts()
        proj_steps(0, xhs, pump, alt_bank=True)
        pump.drain()

        prev_fin = None     # (qb, au_tiles, sums_t) awaiting finalize
        for qb in range(N_QB):
            if qb + 1 < N_QB:
                xhs = proj_prefetch(qb + 1)
                proj_steps(qb + 1, xhs, pump)
            if prev_fin is not None:
                fin_steps(*prev_fin, late)

            n_kc = (qb + 1) * R if causal else N_KC
            slots = n_kc * M_CH
            lslots = max(1, n_kc * (M_CH - 1))
            au_tiles = {}
            sums_t = [sums_pool.tile([97, QB], F32, tag=f"sums{h}",
                                     name=f"sums{h}") for h in (0, 1)]
            for h in (0, 1):
                nc.gpsimd.memset(sums_t[h], 1.0)
            is_last = qb == N_QB - 1
            if is_last:
                eager_fin = make_fin(qb, au_tiles, sums_t)
            for m in range(M_CH):
                if is_last and m == 2:
                    # sums rows for m0/m1 are in flight: reciprocal half 0
                    # and their normalizes overlap the m2/m3 attention.
                    eager_fin[0](0)
                    eager_fin[1](0)
                    eager_fin[1](1)
                pv_t = [pv_ps.tile([65, QB], F32, tag="pv", name=f"pv{hl}")
                        for hl in (0, 1)]

                def emit_pv(pt, c0, kc):
                    for hl in (0, 1):
                        nc.tensor.matmul(
                            pv_t[hl][:, c0:],
                            lhsT=v_aug[:, kc, 2 * m + hl, :],
                            rhs=pt[:, hl, c0:],
                            start=(kc == 0), stop=(kc == n_kc - 1),
                        )

                pending = None   # chunk whose PV is deferred one iteration
                for kc in range(n_kc):
                    # spread pending proj work over the whole loop; the
                    # finalize of qb-1 waits until m >= 1 (its reciprocal
                    # needs qb-1's sums DMAs, which land around m0).
                    pump.run(-(-len(pump.q) // slots))
                    slots -= 1
                    if m >= 1:
                        late.run(-(-len(late.q) // lslots))
                        lslots -= 1

                    r = kc - (n_kc - R)
                    is_diag = causal and r >= 0
                    c0 = KC * r if is_diag and r > 0 else 0
                    sc = sc_ps.tile([P, 2, QB], F32, tag="sc", name="sct")
                    for hl in (0, 1):
                        rows = slice(64 * hl, 64 * hl + 64)
                        nc.tensor.matmul(
                            sc[:, hl, c0:],
                            lhsT=kT[rows, m, kc * KC:(kc + 1) * KC],
                            rhs=qT[rows, m, qb * QB + c0:(qb + 1) * QB],
                            start=True, stop=True,
                        )
                    pt = probs_pool.tile([P, 2, QB], DT, tag="pt", name="pt")
                    nc.scalar.activation(pt[:, :, c0:], sc[:, :, c0:],
                                         AF.Exp, scale=float(SCALE))
                    if is_diag:
                        # only the 128-wide band [c0, c0+KC) is partially
                        # masked; columns beyond it are fully visible
                        for hl in (0, 1):
                            nc.vector.tensor_tensor(
                                pt[:, hl, c0:c0 + KC], pt[:, hl, c0:c0 + KC],
                                dmask[:, r, c0:c0 + KC], ALU.mult)
                    # software pipelining: this chunk's PV is emitted after
                    # the NEXT chunk's scores, so the PE queue never
                    # head-of-line blocks on the exp that PV depends on
                    if pending is not None:
                        emit_pv(*pending)
                    pending = (pt, c0, kc)
                emit_pv(*pending)
                for hl in (0, 1):
                    idx = 2 * m + hl
                    au = aupool.tile([65, QB], F32, tag="au",
                                     name=f"au{idx}")
                    nc.vector.tensor_copy(au, pv_t[hl])
                    au_tiles[idx] = au
                    half, j = divmod(idx, 4)
                    nc.sync.dma_start(sums_t[half][32 * j:32 * j + 1, :],
                                      au[64:65, :])
            pump.drain()
            late.drain()
            prev_fin = (qb, au_tiles, sums_t)

        # last query-block: m0/m1 were normalized eagerly inside the loop
        recip_half, rb_one, op_steps = eager_fin
        recip_half(1)
        rb_one(2)
        rb_one(3)
        op_steps(late, alt_bank=True)
        late.drain()

    nc.compile()
    return nc


def make_consts(S, use_bf16):
    """Host-built 0/1 causal masks for the R diagonal key-chunks."""
    d = _dims(S)
    QB, R = d["QB"], d["R"]
    npdt = _np_dt(use_bf16)
    i = np.arange(P)[:, None]
    j = np.arange(QB)[None, :]
    dmask = np.stack([(i <= j - KC * r) for r in range(R)], 1)
    return dmask.astype(npdt)


def core_inputs(Q, K, V, W_q, b_q, W_k, b_k, W_v, b_v, W_o, b, hg, S, use_bf16,
                proj_fp8=False):
    """Build the per-core input map (host-side slicing/transposition/casts)."""
    npdt = _np_dt(use_bf16)
    d = _dims(S)
    M_CH = d["M_CH"]
    rows = slice(hg * DH, (hg + 1) * DH)

    QB = d["QB"]
    E_CH = D // P
    E2 = E_CH // 2
    if proj_fp8:
        import ml_dtypes
        fp8dt = ml_dtypes.float8_e4m3fn

    def xt(x):
        # [S, D] -> [N_QB, P, E_CH, QB] (bf16) or [N_QB, P, E2, 2, QB] (fp8
        # DoubleRow pairs): transposed/tiled so each block load is one DMA.
        a = np.asarray(x, np.float32).T                   # [D, S]
        if proj_fp8:
            a = a.astype(fp8dt)
            a = a.reshape(E2, 2, P, S // QB, QB).transpose(3, 2, 0, 1, 4)
        else:
            a = a.astype(npdt)
            a = a.reshape(E_CH, P, S // QB, QB).transpose(2, 1, 0, 3)
        return np.ascontiguousarray(a)

    def wt(w):
        # [DH, D] slice -> W^T tiled [P, E_CH, DH] / [P, E2, 2, DH]
        a = np.asarray(w, np.float32).T                   # [D, DH]
        if proj_fp8:
            a = a.astype(fp8dt)
            return np.ascontiguousarray(
                a.reshape(E2, 2, P, DH).transpose(2, 0, 1, 3))
        a = a.astype(npdt)
        return np.ascontiguousarray(
            a.reshape(E_CH, P, DH).transpose(1, 0, 2))

    a_wo = np.asarray(W_o[:, rows], np.float32).T.astype(npdt)  # [DH, D]
    wo_prep = np.ascontiguousarray(
        a_wo.reshape(M_CH, P, D).transpose(1, 0, 2))

    dmask = make_consts(S, use_bf16)
    return {
        "xq_t": xt(Q[b]), "xk_t": xt(K[b]), "xv_t": xt(V[b]),
        "wq_t": wt(W_q[rows]), "wk_t": wt(W_k[rows]), "wv_t": wt(W_v[rows]),
        "wo_t": wo_prep,
        "bq_p": np.ascontiguousarray(
            np.asarray(b_q[rows], np.float32).reshape(M_CH, P).T),
        "bk_p": np.ascontiguousarray(
            np.asarray(b_k[rows], np.float32).reshape(M_CH, P).T),
        "bv_r": np.broadcast_to(
            np.asarray(b_v[rows], np.float32), (P, DH)).copy(),
        "dmask": dmask,
        "ones_c": np.ones((65, 64), np.float32),
        "ones_v": np.ones((P, d["N_KC"], NH_G, 1), npdt),
    }


def _np_reference(Q, K, V, mask, W_q, b_q, W_k, b_k, W_v, b_v, W_o, b_o):
    """Exact numpy fallback for arbitrary masks."""
    q = (Q @ W_q.T + b_q).reshape(B, S_FULL, H, DK).transpose(0, 2, 1, 3)
    k = (K @ W_k.T + b_k).reshape(B, S_FULL, H, DK).transpose(0, 2, 1, 3)
    v = (V @ W_v.T + b_v).reshape(B, S_FULL, H, DK).transpose(0, 2, 1, 3)
    scores = np.einsum("bhqd,bhkd->bhqk", q, k) / np.sqrt(np.float32(DK))
    scores = np.where(mask == 0, np.finfo(np.float32).min, scores)
    scores -= scores.max(-1, keepdims=True)
    probs = np.exp(scores)
    probs /= probs.sum(-1, keepdims=True)
    out = np.einsum("bhqk,bhkd->bhqd", probs, v)
    out = out.transpose(0, 2, 1, 3).reshape(B, S_FULL, D)
    return (out @ W_o.T + b_o).astype(np.float32)


def kernel(Q, K, V, mask, W_q, b_q, W_k, b_k, W_v, b_v, W_o, b_o):
    Q = np.asarray(Q, np.float32)
    K = np.asarray(K, np.float32)
    V = np.asarray(V, np.float32)
    mask = np.asarray(mask)

    m2 = mask.reshape(mask.shape[-2], mask.shape[-1])
    if np.array_equal(m2 != 0, np.tril(np.ones(m2.shape, bool))):
        causal = True
    elif (m2 != 0).all():
        causal = False
    else:
        return _np_reference(Q, K, V, mask, W_q, b_q, W_k, b_k, W_v, b_v,
                             W_o, b_o)

    use_bf16 = os.environ.get("MHA_KERNEL_DTYPE", "bf16") == "bf16"
    proj_fp8 = os.environ.get("MHA_PROJ_FP8", "0") == "1"
    from concourse.bass_utils import run_bass_kernel_spmd

    key = (causal, S_FULL, use_bf16, proj_fp8)
    if key not in _PROG_CACHE:
        _PROG_CACHE[key] = build_program(causal, S_FULL, use_bf16, proj_fp8)
    nc = _PROG_CACHE[key]

    in_maps = []
    for c in range(8):
        b, hg = divmod(c, 2)
        in_maps.append(core_inputs(Q, K, V, W_q, b_q, W_k, b_k, W_v, b_v,
                                   W_o, b, hg, S_FULL, use_bf16, proj_fp8))

    trace = os.environ.get("MHA_KERNEL_TRACE", "0") == "1"
    kw = {}
    if trace:
        kw = {"trace": True,
              "trace_cores": [int(x) for x in os.environ.get(
                  "MHA_TRACE_CORES", "0").split(",")]}
    n_cores = int(os.environ.get("MHA_CORES", "8"))
    res = run_bass_kernel_spmd(nc, in_maps[:n_cores],
                               core_ids=list(range(n_cores)), **kw)
    kernel.last_results = res

    b_o32 = np.asarray(b_o, np.float32)
    out = np.zeros((B, S_FULL, D), np.float32)
    for b in range(B):
        if 2 * b + 1 < n_cores:
            out[b] = (np.asarray(res.results[2 * b]["out_p"], np.float32)
                      + np.asarray(res.results[2 * b + 1]["out_p"],
                                   np.float32) + b_o32[None, :])
    return out


kernel.last_results = None


# revision 57
# speedup vs baseline: 1.0212x; 1.0100x over previous
"""MultiHeadAttention Trainium2 kernel (8 NeuronCores).

Sharding: core c handles batch b = c // 2 and head-group hg = c % 2
(8 of 16 heads, 512 of 1024 model dims). Attention is embarrassingly
parallel over (b, hg); the output projection is computed per head-group
against the matching W_o columns, yielding partial outputs that the host
sums (plus b_o).

Device dataflow (per core), all in "transposed" layouts so no on-device
transposes are ever needed:
  qT = Wq_hg @ Xq^T      [dh=512, S]   (lhsT = Wq_hg^T, rhs = Xq^T)
  kT = Wk_hg @ Xk^T      [dh=512, S]
  v  = Xv @ Wv_hg^T      [S, dh=512]   (+ ones column per head for softmax sums)
  scores_T[k, q] = kT_h^T-matmul, two heads PE-row-tiled concurrently
  probs = exp(scores_T / 8), one ACT instruction per head-pair (PSUM
  bank-pair read) — no max subtraction: scores ~ N(0,1), safe
  causal diag chunks: 0/1 mask multiply (post-exp) + column-trimmed
  matmuls (no memsets)
  attn_T[d, q] (+ sums row) = v^T-matmul over probs, PSUM-accumulated
  normalize: batched reciprocal, broadcast via ones-matmul, multiply
  out_partial = attn^T-matmul with Wo columns

The program is emitted as one fused pipeline: projection of block b+1
and the finalize (normalize + output projection) of query-block b-1 are
interleaved ("pumped") into attention block b's loop so the PE never
drains behind the scalar engine's exp stream.
"""

import os
from collections import deque

import numpy as np

B, S_FULL, D = 4, 2048, 1024
H, DK = 16, 64
NH_G = 8          # heads per core
DH = NH_G * DK    # 512 dims per core
P = 128
KC = 128          # key chunk (PE contraction)
SCALE = 1.0 / np.sqrt(np.float32(DK))

_PROG_CACHE = {}


def _dims(S):
    QB = min(512, S)
    return {
        "S": S, "QB": QB, "N_QB": S // QB, "N_KC": S // KC,
        "R": QB // KC, "E_CH": D // P, "M_CH": DH // P, "O_N": D // 512,
    }


def _np_dt(use_bf16):
    if use_bf16:
        import ml_dtypes
        return ml_dtypes.bfloat16
    return np.float32


class _Pump:
    """Deque of emission closures drained into another loop's gaps."""

    def __init__(self):
        self.q = deque()

    def add(self, fn):
        self.q.append(fn)

    def run(self, n):
        for _ in range(min(n, len(self.q))):
            self.q.popleft()()

    def drain(self):
        while self.q:
            self.q.popleft()()


def build_program(causal, S, use_bf16, proj_fp8=False, debug_dumps=False):
    """Build the single-core Bass/Tile program (same program on all 8 cores)."""
    from contextlib import ExitStack

    import concourse.bass as bass  # noqa: F401
    import concourse.tile as tile
    from concourse import bacc, mybir

    d = _dims(S)
    QB, N_QB, N_KC, R, E_CH, M_CH, O_N = (
        d["QB"], d["N_QB"], d["N_KC"], d["R"], d["E_CH"], d["M_CH"], d["O_N"])
    E2 = E_CH // 2      # 256-deep DoubleRow contraction chunks

    DT = mybir.dt.bfloat16 if use_bf16 else mybir.dt.float32r
    F32 = mybir.dt.float32
    F32R = mybir.dt.float32r
    FP8 = mybir.dt.float8e4
    XDT = FP8 if proj_fp8 else DT
    AF = mybir.ActivationFunctionType
    ALU = mybir.AluOpType
    DROW = mybir.MatmulPerfMode.DoubleRow

    nc = bacc.Bacc("TRN2", target_bir_lowering=False, debug=False)

    NB = S // QB
    SC_B = QB // P      # 128-row chunks per block
    xshape = [NB, P, E2, 2, QB] if proj_fp8 else [NB, P, E_CH, QB]
    wshape = [P, E2, 2, DH] if proj_fp8 else [P, E_CH, DH]
    xq_t = nc.dram_tensor("xq_t", xshape, XDT, kind="ExternalInput").ap()
    xk_t = nc.dram_tensor("xk_t", xshape, XDT, kind="ExternalInput").ap()
    xv_t = nc.dram_tensor("xv_t", xshape, XDT, kind="ExternalInput").ap()
    wq_t = nc.dram_tensor("wq_t", wshape, XDT, kind="ExternalInput").ap()
    wk_t = nc.dram_tensor("wk_t", wshape, XDT, kind="ExternalInput").ap()
    wv_t = nc.dram_tensor("wv_t", wshape, XDT, kind="ExternalInput").ap()
    wo_t = nc.dram_tensor("wo_t", [P, M_CH, D], DT,
                          kind="ExternalInput").ap()
    bq_in = nc.dram_tensor("bq_p", [P, M_CH], F32, kind="ExternalInput").ap()
    bk_in = nc.dram_tensor("bk_p", [P, M_CH], F32, kind="ExternalInput").ap()
    bv_in = nc.dram_tensor("bv_r", [P, DH], F32, kind="ExternalInput").ap()
    dmask_in = nc.dram_tensor("dmask", [P, R, QB], DT,
                              kind="ExternalInput").ap()
    ones_c_in = nc.dram_tensor("ones_c", [65, 64], F32R,
                               kind="ExternalInput").ap()
    ones_v_in = nc.dram_tensor("ones_v", [P, N_KC, NH_G, 1], DT,
                               kind="ExternalInput").ap()
    out_p = nc.dram_tensor("out_p", [S, D], DT, kind="ExternalOutput").ap()

    with tile.TileContext(nc) as tc, ExitStack() as ctx:
        consts = ctx.enter_context(tc.tile_pool(name="consts", bufs=1))
        wpool = ctx.enter_context(tc.tile_pool(name="w", bufs=1))
        qkv = ctx.enter_context(tc.tile_pool(name="qkv", bufs=1))
        xpool = ctx.enter_context(tc.tile_pool(name="xp", bufs=3))
        proj_ps = ctx.enter_context(
            tc.tile_pool(name="pj", bufs=1, space="PSUM"))
        sc_ps = ctx.enter_context(
            tc.tile_pool(name="sc", bufs=2, space="PSUM"))
        pv_ps = ctx.enter_context(
            tc.tile_pool(name="pv", bufs=2, space="PSUM"))
        fin_ps = ctx.enter_context(
            tc.tile_pool(name="fin", bufs=1, space="PSUM"))
        probs_pool = ctx.enter_context(tc.tile_pool(name="probs", bufs=6))
        aupool = ctx.enter_context(tc.tile_pool(name="au", bufs=18))
        attn_pool = ctx.enter_context(tc.tile_pool(name="attn", bufs=9))
        rbpool = ctx.enter_context(tc.tile_pool(name="rbb", bufs=2))
        sums_pool = ctx.enter_context(tc.tile_pool(name="sums", bufs=2))
        outst = ctx.enter_context(tc.tile_pool(name="outst", bufs=3))

        qT = qkv.tile([P, M_CH, S], DT, tag="qT")
        kT = qkv.tile([P, M_CH, S], DT, tag="kT")
        v_aug = qkv.tile([P, N_KC, NH_G, 65], DT, tag="v_aug")

        def load_consts():
            """Emit const/weight DMAs that are not needed immediately; they
            ride the gpsimd DMA ring, parallel to the x loads on sync."""
            nc.gpsimd.dma_start(bq_sb, bq_in)
            nc.gpsimd.dma_start(bk_sb, bk_in)
            nc.gpsimd.dma_start(bv_sb, bv_in)
            nc.gpsimd.dma_start(dmask, dmask_in)
            nc.gpsimd.dma_start(ones_c, ones_c_in)
            if use_bf16:
                nc.gpsimd.memset(v_aug[:, :, :, 64:65], 1.0)
            else:
                nc.gpsimd.dma_start(v_aug[:, :, :, 64:65], ones_v_in)
            w_sb = wpool.tile([P, M_CH, D], DT, tag="wo")
            nc.gpsimd.dma_start(w_sb, wo_t)
            return w_sb

        bq_sb = consts.tile([P, M_CH], F32, tag="bq")
        bk_sb = consts.tile([P, M_CH], F32, tag="bk")
        bv_sb = consts.tile([P, DH], F32, tag="bv")
        dmask = consts.tile([P, R, QB], DT, tag="dmask")
        ones_c = consts.tile([65, 64], F32R, tag="ones_c")

        w_tiles = {}
        wtile_shape = [P, E2, 2, DH] if proj_fp8 else [P, E_CH, DH]
        W_SRC = {"wk": wk_t, "wv": wv_t, "wq": wq_t}

        PJW = max(QB, DH)
        PHASES = (("k", xk_t, "wk"), ("v", xv_t, "wv"), ("q", xq_t, "wq"))

        xtile_shape = [P, E2, 2, QB] if proj_fp8 else [P, E_CH, QB]

        def proj_prefetch(blk, load_w=False):
            """Issue the x (and for block 0, interleaved weight) DMAs so the
            first projection matmul's dependencies sit at the queue head.
            Block 0's k-phase tensors are split per contraction chunk so the
            first matmul starts after ~200KB instead of 1.5MB; the other
            weights ride the gpsimd DMA ring in parallel."""
            xhs = {}
            for phase, x_in, wname in PHASES:
                xblk = xpool.tile(xtile_shape, XDT, tag="x",
                                  name=f"x{phase}{blk}")
                if load_w:
                    # block 0: everything rides the sync queue (hardware
                    # DGE — the gpsimd/scalar queues take the slow software
                    # descriptor path) in exact consumption order. The
                    # k phase is half-split so the first matmul chain's
                    # dependencies clear after ~0.75MB instead of 1.5MB.
                    w_sb = wpool.tile(wtile_shape, XDT, tag=wname, name=wname)
                    if phase == "k":
                        h = wtile_shape[1] // 2
                        for lo in (slice(0, h), slice(h, None)):
                            nc.sync.dma_start(w_sb[:, lo], W_SRC[wname][:, lo])
                            nc.sync.dma_start(xblk[:, lo], x_in[blk, :, lo])
                    else:
                        nc.sync.dma_start(w_sb, W_SRC[wname])
                        nc.sync.dma_start(xblk, x_in[blk])
                    w_tiles[wname] = w_sb
                else:
                    nc.sync.dma_start(xblk, x_in[blk])
                xhs[phase] = xblk
            return xhs

        def proj_steps(blk, xhs, pump, alt_bank=False):
            """Append projection emission steps for s-block `blk`. With
            alt_bank (block 0, before attention exists), alternate chains
            into the idle fin bank so accumulation double-buffers."""
            cnt = {"i": 0}

            def pj_alloc(ph):
                tag = "fin" if alt_bank and cnt["i"] % 2 else "pj"
                pool = fin_ps if tag == "fin" else proj_ps
                cnt["i"] += 1
                ph["ps"] = pool.tile([P, PJW], F32, tag=tag, name="pjt")

            for phase, x_in, wname in PHASES:
                w_sb = w_tiles[wname]
                xh = {"x": xhs[phase]}

                n_mm = E2 if proj_fp8 else E_CH
                if phase in ("q", "k"):
                    b_sb = bq_sb if phase == "q" else bk_sb
                    dstp = qT if phase == "q" else kT
                    for m in range(M_CH):
                        ph = {}

                        def alloc_step(ph=ph):
                            pj_alloc(ph)
                        pump.add(alloc_step)
                        for e in range(n_mm):
                            def mm_step(m=m, e=e, w_sb=w_sb, ph=ph, xh=xh,
                                        n_mm=n_mm):
                                if proj_fp8:
                                    nc.tensor.matmul(
                                        ph["ps"][:, 0:QB],
                                        lhsT=w_sb[:, e, :, m * P:(m + 1) * P],
                                        rhs=xh["x"][:, e],
                                        start=(e == 0), stop=(e == n_mm - 1),
                                        perf_mode=DROW,
                                    )
                                else:
                                    nc.tensor.matmul(
                                        ph["ps"][:, 0:QB],
                                        lhsT=w_sb[:, e, m * P:(m + 1) * P],
                                        rhs=xh["x"][:, e, :],
                                        start=(e == 0), stop=(e == n_mm - 1),
                                    )
                            pump.add(mm_step)

                        def drain_step(m=m, dstp=dstp, b_sb=b_sb, ph=ph):
                            sl = slice(blk * QB, (blk + 1) * QB)
                            nc.vector.tensor_scalar_add(
                                dstp[:, m, sl], ph["ps"][:, 0:QB],
                                b_sb[:, m:m + 1])
                        pump.add(drain_step)
                else:
                    for sc in range(SC_B):
                        ph = {}

                        def alloc_step(ph=ph):
                            pj_alloc(ph)
                        pump.add(alloc_step)
                        for e in range(n_mm):
                            def mm_step(sc=sc, e=e, w_sb=w_sb, ph=ph, xh=xh,
                                        n_mm=n_mm):
                                if proj_fp8:
                                    nc.tensor.matmul(
                                        ph["ps"][:, 0:DH],
                                        lhsT=xh["x"][:, e, :,
                                                     sc * P:(sc + 1) * P],
                                        rhs=w_sb[:, e],
                                        start=(e == 0), stop=(e == n_mm - 1),
                                        perf_mode=DROW,
                                    )
                                else:
                                    nc.tensor.matmul(
                                        ph["ps"][:, 0:DH],
                                        lhsT=xh["x"][:, e, sc * P:(sc + 1) * P],
                                        rhs=w_sb[:, e, :],
                                        start=(e == 0), stop=(e == n_mm - 1),
                                    )
                            pump.add(mm_step)

                        def drain_step(sc=sc, ph=ph):
                            kc = blk * SC_B + sc
                            nc.vector.tensor_tensor(
                                v_aug[:, kc, :, 0:64],
                                ph["ps"][:, 0:DH].rearrange(
                                    "p (h e) -> p h e", h=NH_G),
                                bv_sb.rearrange("p (h e) -> p h e", h=NH_G),
                                ALU.add,
                            )
                        pump.add(drain_step)

        def make_fin(qb, au_tiles, sums_t):
            """Finalize-qb emitters: reciprocal, broadcast+normalize,
            out-projection. Returned closures are composed either via the
            late pump (steady state) or eagerly (last query-block)."""
            st8 = {"recips": [None, None], "attn": {}}

            def recip_half(half):
                # column-chunked so each DVE reciprocal stays ~1us and the
                # attention mask multiplies can slot in between
                rt = sums_pool.tile([97, QB], F32R, tag=f"rec{half}",
                                    name=f"rec{half}")
                with nc.allow_low_precision(
                        reason="softmax denom recip, f32r rounding"):
                    for c in range(0, QB, KC):
                        nc.vector.reciprocal(rt[:, c:c + KC],
                                             sums_t[half][:, c:c + KC])
                st8["recips"][half] = rt

            def rb_one(m):
                attn_m = attn_pool.tile([P, QB], DT, tag="attn",
                                        name="attn_m")
                for hl in (0, 1):
                    idx = 2 * m + hl
                    half, j = divmod(idx, 4)
                    recip65 = rbpool.tile([65, QB], F32R, tag="rbb",
                                          name="recip65")
                    nc.sync.dma_start(
                        recip65[64:65, :],
                        st8["recips"][half][32 * j:32 * j + 1, :])
                    rb = fin_ps.tile([64, QB], F32, tag="fin", name="rb")
                    nc.tensor.matmul(rb, lhsT=ones_c[64:65, :],
                                     rhs=recip65[64:65, :],
                                     start=True, stop=True)
                    nc.vector.tensor_tensor(
                        attn_m[64 * hl:64 * hl + 64, :],
                        au_tiles[idx][0:64, :], rb, ALU.mult)
                st8["attn"][m] = attn_m

            def op_steps(pump, alt_bank=False):
                for i, (ssub, nout) in enumerate(
                        (s, n) for s in range(QB // P) for n in range(O_N)):
                        oh = {}
                        # on the last query-block the proj bank is idle:
                        # alternate into it so out-proj groups double-buffer
                        tag = "pj" if alt_bank and i % 2 else "fin"
                        pool = proj_ps if tag == "pj" else fin_ps

                        def op_mm_step(ssub=ssub, nout=nout, oh=oh,
                                       tag=tag, pool=pool):
                            pso = pool.tile([P, 512], F32, tag=tag,
                                            name="pso")
                            for m in range(M_CH):
                                nc.tensor.matmul(
                                    pso,
                                    lhsT=st8["attn"][m][:,
                                                        ssub * P:(ssub + 1) * P],
                                    rhs=wo_sb[:, m,
                                              nout * 512:(nout + 1) * 512],
                                    start=(m == 0), stop=(m == M_CH - 1),
                                )
                            oh["pso"] = pso
                        pump.add(op_mm_step)

                        def op_st_step(ssub=ssub, nout=nout, oh=oh, i=i,
                                       alt_bank=alt_bank):
                            st = outst.tile([P, 512], DT, tag="st",
                                            name="st")
                            nc.vector.tensor_copy(st, oh["pso"])
                            r0 = qb * QB + ssub * P
                            # last block: sync queue is idle and gpsimd's
                            # software-DGE path would stretch the drain tail
                            eng = (nc.sync if alt_bank
                                   else (nc.gpsimd if i % 2 else nc.sync))
                            eng.dma_start(
                                out_p[r0:r0 + P,
                                      nout * 512:(nout + 1) * 512], st)
                        pump.add(op_st_step)

            return recip_half, rb_one, op_steps

        def fin_steps(qb, au_tiles, sums_t, pump):
            recip_half, rb_one, op_steps = make_fin(qb, au_tiles, sums_t)
            pump.add(lambda: recip_half(0))
            pump.add(lambda: recip_half(1))
            for m in range(M_CH):
                pump.add(lambda m=m: rb_one(m))
            op_steps(pump)

        # ---- fused pipeline ----
        pump = _Pump()      # projection work for the next block
        late = _Pump()      # finalize work for the previous query-block
        xhs = proj_prefetch(0, load_w=True)
        wo_sb = load_consts()
        proj_steps(0, xhs, pump, alt_bank=True)
        pump.drain()

        prev_fin = None     # (qb, au_tiles, sums_t) awaiting finalize
        for qb in range(N_QB):
            if qb + 1 < N_QB:
                xhs = proj_prefetch(qb + 1)
                proj_steps(qb + 1, xhs, pump)
            if prev_fin is not None:
                fin_steps(*prev_fin, late)

            n_kc = (qb + 1) * R if causal else N_KC
            slots = n_kc * M_CH
            lslots = max(1, n_kc * (M_CH - 1))
            au_tiles = {}
            sums_t = [sums_pool.tile([97, QB], F32, tag=f"sums{h}",
                                     name=f"sums{h}") for h in (0, 1)]
            for h in (0, 1):
                nc.gpsimd.memset(sums_t[h], 1.0)
            is_last = qb == N_QB - 1
            if is_last:
                eager_fin = make_fin(qb, au_tiles, sums_t)
            for m in range(M_CH):
                if is_last and m == 2:
                    # sums rows for m0/m1 are in flight: reciprocal half 0
                    # and their normalizes overlap the m2/m3 attention.
                    eager_fin[0](0)
                    eager_fin[1](0)
                    eager_fin[1](1)
                pv_t = [pv_ps.tile([65, QB], F32, tag="pv", name=f"pv{hl}")
                        for hl in (0, 1)]

                def emit_pv(pt, c0, kc):
                    for hl in (0, 1):
                        nc.tensor.matmul(
                            pv_t[hl][:, c0:],
                            lhsT=v_aug[:, kc, 2 * m + hl, :],
                            rhs=pt[:, hl, c0:],
                            start=(kc == 0), stop=(kc == n_kc - 1),
                        )

                pending = None   # chunk whose PV is deferred one iteration
                for kc in range(n_kc):
                    # spread pending proj work over the whole loop; the
                    # finalize of qb-1 waits until m >= 1 (its reciprocal
                    # needs qb-1's sums DMAs, which land around m0).
                    pump.run(-(-len(pump.q) // slots))
                    slots -= 1
                    if m >= 1:
                        late.run(-(-len(late.q) // lslots))
                        lslots -= 1

                    r = kc - (n_kc - R)
                    is_diag = causal and r >= 0
                    c0 = KC * r if is_diag and r > 0 else 0
                    sc = sc_ps.tile([P, 2, QB], F32, tag="sc", name="sct")
                    for hl in (0, 1):
                        rows = slice(64 * hl, 64 * hl + 64)
                        nc.tensor.matmul(
                            sc[:, hl, c0:],
                            lhsT=kT[rows, m, kc * KC:(kc + 1) * KC],
                            rhs=qT[rows, m, qb * QB + c0:(qb + 1) * QB],
                            start=True, stop=True,
                        )
                    pt = probs_pool.tile([P, 2, QB], DT, tag="pt", name="pt")
                    nc.scalar.activation(pt[:, :, c0:], sc[:, :, c0:],
                                         AF.Exp, scale=float(SCALE))
                    if is_diag:
                        # only the 128-wide band [c0, c0+KC) is partially
                        # masked; columns beyond it are fully visible
                        for hl in (0, 1):
                            nc.vector.tensor_tensor(
                                pt[:, hl, c0:c0 + KC], pt[:, hl, c0:c0 + KC],
                                dmask[:, r, c0:c0 + KC], ALU.mult)
                    # software pipelining: this chunk's PV is emitted after
                    # the NEXT chunk's scores, so the PE queue never
                    # head-of-line blocks on the exp that PV depends on
                    if pending is not None:
                        emit_pv(*pending)
                    pending = (pt, c0, kc)
                emit_pv(*pending)
                for hl in (0, 1):
                    idx = 2 * m + hl
                    au = aupool.tile([65, QB], F32, tag="au",
                                     name=f"au{idx}")
                    nc.vector.tensor_copy(au, pv_t[hl])
                    au_tiles[idx] = au
                    half, j = divmod(idx, 4)
                    nc.sync.dma_start(sums_t[half][32 * j:32 * j + 1, :],
                                      au[64:65, :])
            pump.drain()
            late.drain()
            prev_fin = (qb, au_tiles, sums_t)

        # last query-block: m0/m1 were normalized eagerly inside the loop
        recip_half, rb_one, op_steps = eager_fin
        recip_half(1)
        rb_one(2)
        rb_one(3)
        op_steps(late, alt_bank=True)
        late.drain()

    nc.compile()
    return nc


def make_consts(S, use_bf16):
    """Host-built 0/1 causal masks for the R diagonal key-chunks."""
    d = _dims(S)
    QB, R = d["QB"], d["R"]
    npdt = _np_dt(use_bf16)
    i = np.arange(P)[:, None]
    j = np.arange(QB)[None, :]
    dmask = np.stack([(i <= j - KC * r) for r in range(R)], 1)
    return dmask.astype(npdt)


def core_inputs(Q, K, V, W_q, b_q, W_k, b_k, W_v, b_v, W_o, b, hg, S, use_bf16,
                proj_fp8=False):
    """Build the per-core input map (host-side slicing/transposition/casts)."""
    npdt = _np_dt(use_bf16)
    d = _dims(S)
    M_CH = d["M_CH"]
    rows = slice(hg * DH, (hg + 1) * DH)

    QB = d["QB"]
    E_CH = D // P
    E2 = E_CH // 2
    if proj_fp8:
        import ml_dtypes
        fp8dt = ml_dtypes.float8_e4m3fn

    def xt(x):
        # [S, D] -> [N_QB, P, E_CH, QB] (bf16) or [N_QB, P, E2, 2, QB] (fp8
        # DoubleRow pairs): transposed/tiled so each block load is one DMA.
        a = np.asarray(x, np.float32).T                   # [D, S]
        if proj_fp8:
            a = a.astype(fp8dt)
            a = a.reshape(E2, 2, P, S // QB, QB).transpose(3, 2, 0, 1, 4)
        else:
            a = a.astype(npdt)
            a = a.reshape(E_CH, P, S // QB, QB).transpose(2, 1, 0, 3)
        return np.ascontiguousarray(a)

    def wt(w):
        # [DH, D] slice -> W^T tiled [P, E_CH, DH] / [P, E2, 2, DH]
        a = np.asarray(w, np.float32).T                   # [D, DH]
        if proj_fp8:
            a = a.astype(fp8dt)
            return np.ascontiguousarray(
                a.reshape(E2, 2, P, DH).transpose(2, 0, 1, 3))
        a = a.astype(npdt)
        return np.ascontiguousarray(
            a.reshape(E_CH, P, DH).transpose(1, 0, 2))

    a_wo = np.asarray(W_o[:, rows], np.float32).T.astype(npdt)  # [DH, D]
    wo_prep = np.ascontiguousarray(
        a_wo.reshape(M_CH, P, D).transpose(1, 0, 2))

    dmask = make_consts(S, use_bf16)
    return {
        "xq_t": xt(Q[b]), "xk_t": xt(K[b]), "xv_t": xt(V[b]),
        "wq_t": wt(W_q[rows]), "wk_t": wt(W_k[rows]), "wv_t": wt(W_v[rows]),
        "wo_t": wo_prep,
        "bq_p": np.ascontiguousarray(
            np.asarray(b_q[rows], np.float32).reshape(M_CH, P).T),
        "bk_p": np.ascontiguousarray(
            np.asarray(b_k[rows], np.float32).reshape(M_CH, P).T),
        "bv_r": np.broadcast_to(
            np.asarray(b_v[rows], np.float32), (P, DH)).copy(),
        "dmask": dmask,
        "ones_c": np.ones((65, 64), np.float32),
        "ones_v": np.ones((P, d["N_KC"], NH_G, 1), npdt),
    }


def _np_reference(Q, K, V, mask, W_q, b_q, W_k, b_k, W_v, b_v, W_o, b_o):
    """Exact numpy fallback for arbitrary masks."""
    q = (Q @ W_q.T + b_q).reshape(B, S_FULL, H, DK).transpose(0, 2, 1, 3)
    k = (K @ W_k.T + b_k).reshape(B, S_FULL, H, DK).transpose(0, 2, 1, 3)
    v = (V @ W_v.T + b_v).reshape(B, S_FULL, H, DK).transpose(0, 2, 1, 3)
    scores = np.einsum("bhqd,bhkd->bhqk", q, k) / np.sqrt(np.float32(DK))
    scores = np.where(mask == 0, np.finfo(np.float32).min, scores)
    scores -= scores.max(-1, keepdims=True)
    probs = np.exp(scores)
    probs /= probs.sum(-1, keepdims=True)
    out = np.einsum("bhqk,bhkd->bhqd", probs, v)
    out = out.transpose(0, 2, 1, 3).reshape(B, S_FULL, D)
    return (out @ W_o.T + b_o).astype(np.float32)


def kernel(Q, K, V, mask, W_q, b_q, W_k, b_k, W_v, b_v, W_o, b_o):
    Q = np.asarray(Q, np.float32)
    K = np.asarray(K, np.float32)
    V = np.asarray(V, np.float32)
    mask = np.asarray(mask)

    m2 = mask.reshape(mask.shape[-2], mask.shape[-1])
    if np.array_equal(m2 != 0, np.tril(np.ones(m2.shape, bool))):
        causal = True
    elif (m2 != 0).all():
        causal = False
    else:
        return _np_reference(Q, K, V, mask, W_q, b_q, W_k, b_k, W_v, b_v,
                             W_o, b_o)

    use_bf16 = os.environ.get("MHA_KERNEL_DTYPE", "bf16") == "bf16"
    proj_fp8 = os.environ.get("MHA_PROJ_FP8", "0") == "1"
    from concourse.bass_utils import run_bass_kernel_spmd

    key = (causal, S_FULL, use_bf16, proj_fp8)
    if key not in _PROG_CACHE:
        _PROG_CACHE[key] = build_program(causal, S_FULL, use_bf16, proj_fp8)
    nc = _PROG_CACHE[key]

    in_maps = []
    for c in range(8):
        b, hg = divmod(c, 2)
        in_maps.append(core_inputs(Q, K, V, W_q, b_q, W_k, b_k, W_v, b_v,
                                   W_o, b, hg, S_FULL, use_bf16, proj_fp8))

    trace = os.environ.get("MHA_KERNEL_TRACE", "0") == "1"
    kw = {}
    if trace:
        kw = {"trace": True,
              "trace_cores": [int(x) for x in os.environ.get(
                  "MHA_TRACE_CORES", "0").split(",")]}
    n_cores = int(os.environ.get("MHA_CORES", "8"))
    res = run_bass_kernel_spmd(nc, in_maps[:n_cores],
                               core_ids=list(range(n_cores)), **kw)
    kernel.last_results = res

    b_o32 = np.asarray(b_o, np.float32)
    out = np.zeros((B, S_FULL, D), np.float32)
    for b in range(B):
        if 2 * b + 1 < n_cores:
            out[b] = (np.asarray(res.results[2 * b]["out_p"], np.float32)
                      + np.asarray(res.results[2 * b + 1]["out_p"],
                                   np.float32) + b_o32[None, :])
    return out


kernel.last_results = None


# revision 58
# speedup vs baseline: 1.0417x; 1.0201x over previous
"""MultiHeadAttention Trainium2 kernel (8 NeuronCores).

Sharding: core c handles batch b = c // 2 and head-group hg = c % 2
(8 of 16 heads, 512 of 1024 model dims). Attention is embarrassingly
parallel over (b, hg); the output projection is computed per head-group
against the matching W_o columns, yielding partial outputs that the host
sums (plus b_o).

Device dataflow (per core), all in "transposed" layouts so no on-device
transposes are ever needed:
  qT = Wq_hg @ Xq^T      [dh=512, S]   (lhsT = Wq_hg^T, rhs = Xq^T)
  kT = Wk_hg @ Xk^T      [dh=512, S]
  v  = Xv @ Wv_hg^T      [S, dh=512]   (+ ones column per head for softmax sums)
  scores_T[k, q] = kT_h^T-matmul, two heads PE-row-tiled concurrently
  probs = exp(scores_T / 8), one ACT instruction per head-pair (PSUM
  bank-pair read) — no max subtraction: scores ~ N(0,1), safe
  causal diag chunks: 0/1 mask multiply (post-exp) + column-trimmed
  matmuls (no memsets)
  attn_T[d, q] (+ sums row) = v^T-matmul over probs, PSUM-accumulated
  normalize: batched reciprocal, broadcast via ones-matmul, multiply
  out_partial = attn^T-matmul with Wo columns

The program is emitted as one fused pipeline: projection of block b+1
and the finalize (normalize + output projection) of query-block b-1 are
interleaved ("pumped") into attention block b's loop so the PE never
drains behind the scalar engine's exp stream.
"""

import os
from collections import deque

import numpy as np

B, S_FULL, D = 4, 2048, 1024
H, DK = 16, 64
NH_G = 8          # heads per core
DH = NH_G * DK    # 512 dims per core
P = 128
KC = 128          # key chunk (PE contraction)
SCALE = 1.0 / np.sqrt(np.float32(DK))

_PROG_CACHE = {}


def _dims(S):
    QB = min(512, S)
    return {
        "S": S, "QB": QB, "N_QB": S // QB, "N_KC": S // KC,
        "R": QB // KC, "E_CH": D // P, "M_CH": DH // P, "O_N": D // 512,
    }


def _np_dt(use_bf16):
    if use_bf16:
        import ml_dtypes
        return ml_dtypes.bfloat16
    return np.float32


class _Pump:
    """Deque of emission closures drained into another loop's gaps."""

    def __init__(self):
        self.q = deque()

    def add(self, fn):
        self.q.append(fn)

    def run(self, n):
        for _ in range(min(n, len(self.q))):
            self.q.popleft()()

    def drain(self):
        while self.q:
            self.q.popleft()()


def build_program(causal, S, use_bf16, proj_fp8=False, debug_dumps=False):
    """Build the single-core Bass/Tile program (same program on all 8 cores)."""
    from contextlib import ExitStack

    import concourse.bass as bass  # noqa: F401
    import concourse.tile as tile
    from concourse import bacc, mybir

    d = _dims(S)
    QB, N_QB, N_KC, R, E_CH, M_CH, O_N = (
        d["QB"], d["N_QB"], d["N_KC"], d["R"], d["E_CH"], d["M_CH"], d["O_N"])
    E2 = E_CH // 2      # 256-deep DoubleRow contraction chunks

    DT = mybir.dt.bfloat16 if use_bf16 else mybir.dt.float32r
    F32 = mybir.dt.float32
    F32R = mybir.dt.float32r
    FP8 = mybir.dt.float8e4
    XDT = FP8 if proj_fp8 else DT
    AF = mybir.ActivationFunctionType
    ALU = mybir.AluOpType
    DROW = mybir.MatmulPerfMode.DoubleRow

    nc = bacc.Bacc("TRN2", target_bir_lowering=False, debug=False)

    NB = S // QB
    SC_B = QB // P      # 128-row chunks per block
    xshape = [NB, P, E2, 2, QB] if proj_fp8 else [NB, P, E_CH, QB]
    wshape = [P, E2, 2, DH] if proj_fp8 else [P, E_CH, DH]
    xq_t = nc.dram_tensor("xq_t", xshape, XDT, kind="ExternalInput").ap()
    xk_t = nc.dram_tensor("xk_t", xshape, XDT, kind="ExternalInput").ap()
    xv_t = nc.dram_tensor("xv_t", xshape, XDT, kind="ExternalInput").ap()
    wq_t = nc.dram_tensor("wq_t", wshape, XDT, kind="ExternalInput").ap()
    wk_t = nc.dram_tensor("wk_t", wshape, XDT, kind="ExternalInput").ap()
    wv_t = nc.dram_tensor("wv_t", wshape, XDT, kind="ExternalInput").ap()
    wo_t = nc.dram_tensor("wo_t", [P, M_CH, D], DT,
                          kind="ExternalInput").ap()
    bq_in = nc.dram_tensor("bq_p", [P, M_CH], F32, kind="ExternalInput").ap()
    bk_in = nc.dram_tensor("bk_p", [P, M_CH], F32, kind="ExternalInput").ap()
    bv_in = nc.dram_tensor("bv_r", [P, DH], F32, kind="ExternalInput").ap()
    dmask_in = nc.dram_tensor("dmask", [P, R, QB], DT,
                              kind="ExternalInput").ap()
    ones_c_in = nc.dram_tensor("ones_c", [65, 64], F32R,
                               kind="ExternalInput").ap()
    ones_v_in = nc.dram_tensor("ones_v", [P, N_KC, NH_G, 1], DT,
                               kind="ExternalInput").ap()
    out_p = nc.dram_tensor("out_p", [S, D], DT, kind="ExternalOutput").ap()

    with tile.TileContext(nc) as tc, ExitStack() as ctx:
        consts = ctx.enter_context(tc.tile_pool(name="consts", bufs=1))
        wpool = ctx.enter_context(tc.tile_pool(name="w", bufs=1))
        qkv = ctx.enter_context(tc.tile_pool(name="qkv", bufs=1))
        xpool = ctx.enter_context(tc.tile_pool(name="xp", bufs=3))
        proj_ps = ctx.enter_context(
            tc.tile_pool(name="pj", bufs=1, space="PSUM"))
        sc_ps = ctx.enter_context(
            tc.tile_pool(name="sc", bufs=2, space="PSUM"))
        pv_ps = ctx.enter_context(
            tc.tile_pool(name="pv", bufs=2, space="PSUM"))
        fin_ps = ctx.enter_context(
            tc.tile_pool(name="fin", bufs=1, space="PSUM"))
        probs_pool = ctx.enter_context(tc.tile_pool(name="probs", bufs=6))
        aupool = ctx.enter_context(tc.tile_pool(name="au", bufs=18))
        attn_pool = ctx.enter_context(tc.tile_pool(name="attn", bufs=9))
        rbpool = ctx.enter_context(tc.tile_pool(name="rbb", bufs=2))
        sums_pool = ctx.enter_context(tc.tile_pool(name="sums", bufs=2))
        outst = ctx.enter_context(tc.tile_pool(name="outst", bufs=3))

        qT = qkv.tile([P, M_CH, S], DT, tag="qT")
        kT = qkv.tile([P, M_CH, S], DT, tag="kT")
        v_aug = qkv.tile([P, N_KC, NH_G, 65], DT, tag="v_aug")

        def load_consts():
            """Emit const/weight DMAs that are not needed immediately; they
            ride the gpsimd DMA ring, parallel to the x loads on sync."""
            nc.gpsimd.dma_start(bq_sb, bq_in)
            nc.gpsimd.dma_start(bk_sb, bk_in)
            nc.gpsimd.dma_start(bv_sb, bv_in)
            nc.gpsimd.dma_start(dmask, dmask_in)
            nc.gpsimd.dma_start(ones_c, ones_c_in)
            if use_bf16:
                nc.gpsimd.memset(v_aug[:, :, :, 64:65], 1.0)
            else:
                nc.gpsimd.dma_start(v_aug[:, :, :, 64:65], ones_v_in)
            w_sb = wpool.tile([P, M_CH, D], DT, tag="wo")
            nc.gpsimd.dma_start(w_sb, wo_t)
            return w_sb

        bq_sb = consts.tile([P, M_CH], F32, tag="bq")
        bk_sb = consts.tile([P, M_CH], F32, tag="bk")
        bv_sb = consts.tile([P, DH], F32, tag="bv")
        dmask = consts.tile([P, R, QB], DT, tag="dmask")
        ones_c = consts.tile([65, 64], F32R, tag="ones_c")

        w_tiles = {}
        wtile_shape = [P, E2, 2, DH] if proj_fp8 else [P, E_CH, DH]
        W_SRC = {"wk": wk_t, "wv": wv_t, "wq": wq_t}

        PJW = max(QB, DH)
        PHASES = (("k", xk_t, "wk"), ("v", xv_t, "wv"), ("q", xq_t, "wq"))

        xtile_shape = [P, E2, 2, QB] if proj_fp8 else [P, E_CH, QB]

        def proj_prefetch(blk, load_w=False):
            """Issue the x (and for block 0, interleaved weight) DMAs so the
            first projection matmul's dependencies sit at the queue head.
            Block 0's k-phase tensors are split per contraction chunk so the
            first matmul starts after ~200KB instead of 1.5MB; the other
            weights ride the gpsimd DMA ring in parallel."""
            xhs = {}
            for phase, x_in, wname in PHASES:
                xblk = xpool.tile(xtile_shape, XDT, tag="x",
                                  name=f"x{phase}{blk}")
                if load_w:
                    # block 0: everything rides the sync queue (hardware
                    # DGE — the gpsimd/scalar queues take the slow software
                    # descriptor path) in exact consumption order
                    w_sb = wpool.tile(wtile_shape, XDT, tag=wname, name=wname)
                    nc.sync.dma_start(w_sb, W_SRC[wname])
                    w_tiles[wname] = w_sb
                nc.sync.dma_start(xblk, x_in[blk])
                xhs[phase] = xblk
            return xhs

        def proj_steps(blk, xhs, pump, alt_bank=False):
            """Append projection emission steps for s-block `blk`. With
            alt_bank (block 0, before attention exists), alternate chains
            into the idle fin bank so accumulation double-buffers."""
            cnt = {"i": 0}

            def pj_alloc(ph):
                tag = "fin" if alt_bank and cnt["i"] % 2 else "pj"
                pool = fin_ps if tag == "fin" else proj_ps
                cnt["i"] += 1
                ph["ps"] = pool.tile([P, PJW], F32, tag=tag, name="pjt")

            for phase, x_in, wname in PHASES:
                w_sb = w_tiles[wname]
                xh = {"x": xhs[phase]}

                n_mm = E2 if proj_fp8 else E_CH
                if phase in ("q", "k"):
                    b_sb = bq_sb if phase == "q" else bk_sb
                    dstp = qT if phase == "q" else kT
                    for m in range(M_CH):
                        ph = {}

                        def alloc_step(ph=ph):
                            pj_alloc(ph)
                        pump.add(alloc_step)
                        for e in range(n_mm):
                            def mm_step(m=m, e=e, w_sb=w_sb, ph=ph, xh=xh,
                                        n_mm=n_mm):
                                if proj_fp8:
                                    nc.tensor.matmul(
                                        ph["ps"][:, 0:QB],
                                        lhsT=w_sb[:, e, :, m * P:(m + 1) * P],
                                        rhs=xh["x"][:, e],
                                        start=(e == 0), stop=(e == n_mm - 1),
                                        perf_mode=DROW,
                                    )
                                else:
                                    nc.tensor.matmul(
                                        ph["ps"][:, 0:QB],
                                        lhsT=w_sb[:, e, m * P:(m + 1) * P],
                                        rhs=xh["x"][:, e, :],
                                        start=(e == 0), stop=(e == n_mm - 1),
                                    )
                            pump.add(mm_step)

                        def drain_step(m=m, dstp=dstp, b_sb=b_sb, ph=ph):
                            sl = slice(blk * QB, (blk + 1) * QB)
                            nc.vector.tensor_scalar_add(
                                dstp[:, m, sl], ph["ps"][:, 0:QB],
                                b_sb[:, m:m + 1])
                        pump.add(drain_step)
                else:
                    for sc in range(SC_B):
                        ph = {}

                        def alloc_step(ph=ph):
                            pj_alloc(ph)
                        pump.add(alloc_step)
                        for e in range(n_mm):
                            def mm_step(sc=sc, e=e, w_sb=w_sb, ph=ph, xh=xh,
                                        n_mm=n_mm):
                                if proj_fp8:
                                    nc.tensor.matmul(
                                        ph["ps"][:, 0:DH],
                                        lhsT=xh["x"][:, e, :,
                                                     sc * P:(sc + 1) * P],
                                        rhs=w_sb[:, e],
                                        start=(e == 0), stop=(e == n_mm - 1),
                                        perf_mode=DROW,
                                    )
                                else:
                                    nc.tensor.matmul(
                                        ph["ps"][:, 0:DH],
                                        lhsT=xh["x"][:, e, sc * P:(sc + 1) * P],
                                        rhs=w_sb[:, e, :],
                                        start=(e == 0), stop=(e == n_mm - 1),
                                    )
                            pump.add(mm_step)

                        def drain_step(sc=sc, ph=ph):
                            kc = blk * SC_B + sc
                            nc.vector.tensor_tensor(
                                v_aug[:, kc, :, 0:64],
                                ph["ps"][:, 0:DH].rearrange(
                                    "p (h e) -> p h e", h=NH_G),
                                bv_sb.rearrange("p (h e) -> p h e", h=NH_G),
                                ALU.add,
                            )
                        pump.add(drain_step)

        def make_fin(qb, au_tiles, sums_t):
            """Finalize-qb emitters: reciprocal, broadcast+normalize,
            out-projection. Returned closures are composed either via the
            late pump (steady state) or eagerly (last query-block)."""
            st8 = {"recips": [None, None], "attn": {}}

            def recip_half(half):
                # column-chunked so each DVE reciprocal stays ~1us and the
                # attention mask multiplies can slot in between
                rt = sums_pool.tile([97, QB], F32R, tag=f"rec{half}",
                                    name=f"rec{half}")
                with nc.allow_low_precision(
                        reason="softmax denom recip, f32r rounding"):
                    for c in range(0, QB, KC):
                        nc.vector.reciprocal(rt[:, c:c + KC],
                                             sums_t[half][:, c:c + KC])
                st8["recips"][half] = rt

            def rb_one(m):
                attn_m = attn_pool.tile([P, QB], DT, tag="attn",
                                        name="attn_m")
                for hl in (0, 1):
                    idx = 2 * m + hl
                    half, j = divmod(idx, 4)
                    recip65 = rbpool.tile([65, QB], F32R, tag="rbb",
                                          name="recip65")
                    nc.sync.dma_start(
                        recip65[64:65, :],
                        st8["recips"][half][32 * j:32 * j + 1, :])
                    rb = fin_ps.tile([64, QB], F32, tag="fin", name="rb")
                    nc.tensor.matmul(rb, lhsT=ones_c[64:65, :],
                                     rhs=recip65[64:65, :],
                                     start=True, stop=True)
                    nc.vector.tensor_tensor(
                        attn_m[64 * hl:64 * hl + 64, :],
                        au_tiles[idx][0:64, :], rb, ALU.mult)
                st8["attn"][m] = attn_m

            def op_steps(pump, alt_bank=False):
                for i, (ssub, nout) in enumerate(
                        (s, n) for s in range(QB // P) for n in range(O_N)):
                        oh = {}
                        # on the last query-block the proj bank is idle:
                        # alternate into it so out-proj groups double-buffer
                        tag = "pj" if alt_bank and i % 2 else "fin"
                        pool = proj_ps if tag == "pj" else fin_ps

                        def op_mm_step(ssub=ssub, nout=nout, oh=oh,
                                       tag=tag, pool=pool):
                            pso = pool.tile([P, 512], F32, tag=tag,
                                            name="pso")
                            for m in range(M_CH):
                                nc.tensor.matmul(
                                    pso,
                                    lhsT=st8["attn"][m][:,
                                                        ssub * P:(ssub + 1) * P],
                                    rhs=wo_sb[:, m,
                                              nout * 512:(nout + 1) * 512],
                                    start=(m == 0), stop=(m == M_CH - 1),
                                )
                            oh["pso"] = pso
                        pump.add(op_mm_step)

                        def op_st_step(ssub=ssub, nout=nout, oh=oh, i=i,
                                       alt_bank=alt_bank):
                            st = outst.tile([P, 512], DT, tag="st",
                                            name="st")
                            nc.vector.tensor_copy(st, oh["pso"])
                            r0 = qb * QB + ssub * P
                            # last block: sync queue is idle and gpsimd's
                            # software-DGE path would stretch the drain tail
                            eng = (nc.sync if alt_bank
                                   else (nc.gpsimd if i % 2 else nc.sync))
                            eng.dma_start(
                                out_p[r0:r0 + P,
                                      nout * 512:(nout + 1) * 512], st)
                        pump.add(op_st_step)

            return recip_half, rb_one, op_steps

        def fin_steps(qb, au_tiles, sums_t, pump):
            recip_half, rb_one, op_steps = make_fin(qb, au_tiles, sums_t)
            pump.add(lambda: recip_half(0))
            pump.add(lambda: recip_half(1))
            for m in range(M_CH):
                pump.add(lambda m=m: rb_one(m))
            op_steps(pump)

        # ---- fused pipeline ----
        pump = _Pump()      # projection work for the next block
        late = _Pump()      # finalize work for the previous query-block
        xhs = proj_prefetch(0, load_w=True)
        wo_sb = load_consts()
        proj_steps(0, xhs, pump, alt_bank=True)
        pump.drain()

        prev_fin = None     # (qb, au_tiles, sums_t) awaiting finalize
        for qb in range(N_QB):
            if qb + 1 < N_QB:
                xhs = proj_prefetch(qb + 1)
                proj_steps(qb + 1, xhs, pump)
            if prev_fin is not None:
                fin_steps(*prev_fin, late)

            n_kc = (qb + 1) * R if causal else N_KC
            slots = n_kc * M_CH
            lslots = max(1, n_kc * (M_CH - 1))
            au_tiles = {}
            sums_t = [sums_pool.tile([97, QB], F32, tag=f"sums{h}",
                                     name=f"sums{h}") for h in (0, 1)]
            for h in (0, 1):
                nc.gpsimd.memset(sums_t[h], 1.0)
            is_last = qb == N_QB - 1
            if is_last:
                eager_fin = make_fin(qb, au_tiles, sums_t)
            for m in range(M_CH):
                if is_last and m == 2:
                    # sums rows for m0/m1 are in flight: reciprocal half 0
                    # and their normalizes overlap the m2/m3 attention.
                    eager_fin[0](0)
                    eager_fin[1](0)
                    eager_fin[1](1)
                pv_t = [pv_ps.tile([65, QB], F32, tag="pv", name=f"pv{hl}")
                        for hl in (0, 1)]

                def emit_pv(pt, c0, kc):
                    for hl in (0, 1):
                        nc.tensor.matmul(
                            pv_t[hl][:, c0:],
                            lhsT=v_aug[:, kc, 2 * m + hl, :],
                            rhs=pt[:, hl, c0:],
                            start=(kc == 0), stop=(kc == n_kc - 1),
                        )

                pending = None   # chunk whose PV is deferred one iteration
                for kc in range(n_kc):
                    # spread pending proj work over the whole loop; the
                    # finalize of qb-1 waits until m >= 1 (its reciprocal
                    # needs qb-1's sums DMAs, which land around m0).
                    pump.run(-(-len(pump.q) // slots))
                    slots -= 1
                    if m >= 1:
                        late.run(-(-len(late.q) // lslots))
                        lslots -= 1

                    r = kc - (n_kc - R)
                    is_diag = causal and r >= 0
                    c0 = KC * r if is_diag and r > 0 else 0
                    sc = sc_ps.tile([P, 2, QB], F32, tag="sc", name="sct")
                    for hl in (0, 1):
                        rows = slice(64 * hl, 64 * hl + 64)
                        nc.tensor.matmul(
                            sc[:, hl, c0:],
                            lhsT=kT[rows, m, kc * KC:(kc + 1) * KC],
                            rhs=qT[rows, m, qb * QB + c0:(qb + 1) * QB],
                            start=True, stop=True,
                        )
                    pt = probs_pool.tile([P, 2, QB], DT, tag="pt", name="pt")
                    nc.scalar.activation(pt[:, :, c0:], sc[:, :, c0:],
                                         AF.Exp, scale=float(SCALE))
                    if is_diag:
                        # only the 128-wide band [c0, c0+KC) is partially
                        # masked; columns beyond it are fully visible
                        for hl in (0, 1):
                            nc.vector.tensor_tensor(
                                pt[:, hl, c0:c0 + KC], pt[:, hl, c0:c0 + KC],
                                dmask[:, r, c0:c0 + KC], ALU.mult)
                    # software pipelining: this chunk's PV is emitted after
                    # the NEXT chunk's scores, so the PE queue never
                    # head-of-line blocks on the exp that PV depends on
                    if pending is not None:
                        emit_pv(*pending)
                    pending = (pt, c0, kc)
                emit_pv(*pending)
                for hl in (0, 1):
                    idx = 2 * m + hl
                    au = aupool.tile([65, QB], F32, tag="au",
                                     name=f"au{idx}")
                    nc.vector.tensor_copy(au, pv_t[hl])
                    au_tiles[idx] = au
                    half, j = divmod(idx, 4)
                    nc.sync.dma_start(sums_t[half][32 * j:32 * j + 1, :],
                                      au[64:65, :])
            pump.drain()
            late.drain()
            prev_fin = (qb, au_tiles, sums_t)

        # last query-block: m0/m1 were normalized eagerly inside the loop
        recip_half, rb_one, op_steps = eager_fin
        recip_half(1)
        rb_one(2)
        rb_one(3)
        op_steps(late, alt_bank=True)
        late.drain()

    nc.compile()
    return nc


def make_consts(S, use_bf16):
    """Host-built 0/1 causal masks for the R diagonal key-chunks."""
    d = _dims(S)
    QB, R = d["QB"], d["R"]
    npdt = _np_dt(use_bf16)
    i = np.arange(P)[:, None]
    j = np.arange(QB)[None, :]
    dmask = np.stack([(i <= j - KC * r) for r in range(R)], 1)
    return dmask.astype(npdt)


def core_inputs(Q, K, V, W_q, b_q, W_k, b_k, W_v, b_v, W_o, b, hg, S, use_bf16,
                proj_fp8=False):
    """Build the per-core input map (host-side slicing/transposition/casts)."""
    npdt = _np_dt(use_bf16)
    d = _dims(S)
    M_CH = d["M_CH"]
    rows = slice(hg * DH, (hg + 1) * DH)

    QB = d["QB"]
    E_CH = D // P
    E2 = E_CH // 2
    if proj_fp8:
        import ml_dtypes
        fp8dt = ml_dtypes.float8_e4m3fn

    def xt(x):
        # [S, D] -> [N_QB, P, E_CH, QB] (bf16) or [N_QB, P, E2, 2, QB] (fp8
        # DoubleRow pairs): transposed/tiled so each block load is one DMA.
        a = np.asarray(x, np.float32).T                   # [D, S]
        if proj_fp8:
            a = a.astype(fp8dt)
            a = a.reshape(E2, 2, P, S // QB, QB).transpose(3, 2, 0, 1, 4)
        else:
            a = a.astype(npdt)
            a = a.reshape(E_CH, P, S // QB, QB).transpose(2, 1, 0, 3)
        return np.ascontiguousarray(a)

    def wt(w):
        # [DH, D] slice -> W^T tiled [P, E_CH, DH] / [P, E2, 2, DH]
        a = np.asarray(w, np.float32).T                   # [D, DH]
        if proj_fp8:
            a = a.astype(fp8dt)
            return np.ascontiguousarray(
                a.reshape(E2, 2, P, DH).transpose(2, 0, 1, 3))
        a = a.astype(npdt)
        return np.ascontiguousarray(
            a.reshape(E_CH, P, DH).transpose(1, 0, 2))

    a_wo = np.asarray(W_o[:, rows], np.float32).T.astype(npdt)  # [DH, D]
    wo_prep = np.ascontiguousarray(
        a_wo.reshape(M_CH, P, D).transpose(1, 0, 2))

    dmask = make_consts(S, use_bf16)
    return {
        "xq_t": xt(Q[b]), "xk_t": xt(K[b]), "xv_t": xt(V[b]),
        "wq_t": wt(W_q[rows]), "wk_t": wt(W_k[rows]), "wv_t": wt(W_v[rows]),
        "wo_t": wo_prep,
        "bq_p": np.ascontiguousarray(
            np.asarray(b_q[rows], np.float32).reshape(M_CH, P).T),
        "bk_p": np.ascontiguousarray(
            np.asarray(b_k[rows], np.float32).reshape(M_CH, P).T),
        "bv_r": np.broadcast_to(
            np.asarray(b_v[rows], np.float32), (P, DH)).copy(),
        "dmask": dmask,
        "ones_c": np.ones((65, 64), np.float32),
        "ones_v": np.ones((P, d["N_KC"], NH_G, 1), npdt),
    }


def _np_reference(Q, K, V, mask, W_q, b_q, W_k, b_k, W_v, b_v, W_o, b_o):
    """Exact numpy fallback for arbitrary masks."""
    q = (Q @ W_q.T + b_q).reshape(B, S_FULL, H, DK).transpose(0, 2, 1, 3)
    k = (K @ W_k.T + b_k).reshape(B, S_FULL, H, DK).transpose(0, 2, 1, 3)
    v = (V @ W_v.T + b_v).reshape(B, S_FULL, H, DK).transpose(0, 2, 1, 3)
    scores = np.einsum("bhqd,bhkd->bhqk", q, k) / np.sqrt(np.float32(DK))
    scores = np.where(mask == 0, np.finfo(np.float32).min, scores)
    scores -= scores.max(-1, keepdims=True)
    probs = np.exp(scores)
    probs /= probs.sum(-1, keepdims=True)
    out = np.einsum("bhqk,bhkd->bhqd", probs, v)
    out = out.transpose(0, 2, 1, 3).reshape(B, S_FULL, D)
    return (out @ W_o.T + b_o).astype(np.float32)


def kernel(Q, K, V, mask, W_q, b_q, W_k, b_k, W_v, b_v, W_o, b_o):
    Q = np.asarray(Q, np.float32)
    K = np.asarray(K, np.float32)
    V = np.asarray(V, np.float32)
    mask = np.asarray(mask)

    m2 = mask.reshape(mask.shape[-2], mask.shape[-1])
    if np.array_equal(m2 != 0, np.tril(np.ones(m2.shape, bool))):
        causal = True
    elif (m2 != 0).all():
        causal = False
    else:
        return _np_reference(Q, K, V, mask, W_q, b_q, W_k, b_k, W_v, b_v,
                             W_o, b_o)

    use_bf16 = os.environ.get("MHA_KERNEL_DTYPE", "bf16") == "bf16"
    proj_fp8 = os.environ.get("MHA_PROJ_FP8", "0") == "1"
    from concourse.bass_utils import run_bass_kernel_spmd

    key = (causal, S_FULL, use_bf16, proj_fp8)
    if key not in _PROG_CACHE:
        _PROG_CACHE[key] = build_program(causal, S_FULL, use_bf16, proj_fp8)
    nc = _PROG_CACHE[key]

    in_maps = []
    for c in range(8):
        b, hg = divmod(c, 2)
        in_maps.append(core_inputs(Q, K, V, W_q, b_q, W_k, b_k, W_v, b_v,
                                   W_o, b, hg, S_FULL, use_bf16, proj_fp8))

    trace = os.environ.get("MHA_KERNEL_TRACE", "0") == "1"
    kw = {}
    if trace:
        kw = {"trace": True,
              "trace_cores": [int(x) for x in os.environ.get(
                  "MHA_TRACE_CORES", "0").split(",")]}
    n_cores = int(os.environ.get("MHA_CORES", "8"))
    res = run_bass_kernel_spmd(nc, in_maps[:n_cores],
                               core_ids=list(range(n_cores)), **kw)
    kernel.last_results = res

    b_o32 = np.asarray(b_o, np.float32)
    out = np.zeros((B, S_FULL, D), np.float32)
    for b in range(B):
        if 2 * b + 1 < n_cores:
            out[b] = (np.asarray(res.results[2 * b]["out_p"], np.float32)
                      + np.asarray(res.results[2 * b + 1]["out_p"],
                                   np.float32) + b_o32[None, :])
    return out


kernel.last_results = None
